# revision 1
# baseline (speedup 1.0000x reference)
"""LITv1 transformer block on 8 TRN2 NeuronCores, data-parallel over batch.

Layout strategy (per core, 8 batches x 256 tokens):
- token-major residual stream + LayerNorm (bn_stats), fp32 exact
- feature-major activations for matmuls (PE transposes of LN outputs)
- fp32r matmuls everywhere (N>=256 -> full PE speed, ~13-bit mantissa)
- transposed softmax: S^T = K^T.T @ Q^T, exp without max-subtraction
  (scores ~N(0,1)), dense bias table exp(bias) precomputed on host,
  softmax denominator via an appended ones-column in V, normalization by
  K=1 ones-matmul broadcast + reciprocal + multiply.
"""
import sys

import numpy as np

sys.path.insert(0, "/opt/trn_rl_repo")

import concourse.bass as bass  # noqa: E402
import concourse.mybir as mybir  # noqa: E402
import concourse.tile as tile  # noqa: E402
from concourse import bacc  # noqa: E402
from concourse.bass_utils import run_bass_kernel_spmd  # noqa: E402
from concourse.masks import make_identity  # noqa: E402

F32 = mybir.dt.float32
F32R = mybir.dt.float32r
AF = mybir.ActivationFunctionType
ALU = mybir.AluOpType

B, N, C = 64, 256, 1024
H, DH = 16, 64
DFF = 4 * C
NCORES = 8
BLOC = B // NCORES          # 8 batches per core
TOK = BLOC * N              # 2048 tokens per core
KC = C // 128               # 8 contraction chunks


def build():
    nc = bacc.Bacc("TRN2")
    x_d = nc.dram_tensor("x", [TOK, C], F32, kind="ExternalInput")
    wqkv_d = nc.dram_tensor("wqkv", [C, 3 * C], F32R, kind="ExternalInput")
    wproj_d = nc.dram_tensor("wproj", [C, C], F32R, kind="ExternalInput")
    wfc1_d = nc.dram_tensor("wfc1", [C, DFF], F32R, kind="ExternalInput")
    wfc2_d = nc.dram_tensor("wfc2", [DFF, C], F32R, kind="ExternalInput")
    expb_d = nc.dram_tensor("expb", [2, 128, H, N], F32R, kind="ExternalInput")
    y_d = nc.dram_tensor("y", [TOK, C], F32, kind="ExternalOutput")

    with tile.TileContext(nc) as tc:
        with (
            tc.tile_pool(name="consts", bufs=1) as consts,
            tc.tile_pool(name="dram", bufs=1, space="DRAM") as dpool,
        ):
            ident_f = consts.tile([128, 128], F32)
            make_identity(nc, ident_f)
            ident = consts.tile([128, 128], F32R)
            nc.vector.tensor_copy(ident, ident_f)
            ones_f = consts.tile([128, 64], F32)
            nc.vector.memset(ones_f, 1.0)
            ones_r = consts.tile([128, 64], F32R)
            nc.vector.tensor_copy(ones_r, ones_f)
            eps_sb = consts.tile([128, 1], F32)
            nc.vector.memset(eps_sb, 1e-5)

            r1_dram = dpool.tile([TOK, C], F32)

            # ---------------- Phase A: attention + proj + residual ----------
            with (
                tc.tile_pool(name="paw", bufs=1) as paw,
                tc.tile_pool(name="pa", bufs=2) as pa,
                tc.tile_pool(name="pa1", bufs=1) as pa1,
                tc.tile_pool(name="paw2", bufs=2) as paw2,
                tc.tile_pool(name="pab", bufs=1) as pab,
                tc.tile_pool(name="psQ", bufs=2, space="PSUM") as psQ,
                tc.tile_pool(name="psV", bufs=2, space="PSUM") as psV,
                tc.tile_pool(name="psS", bufs=1, space="PSUM") as psS,
                tc.tile_pool(name="psO", bufs=1, space="PSUM") as psO,
                tc.tile_pool(name="psBC", bufs=1, space="PSUM") as psBC,
                tc.tile_pool(name="psT", bufs=1, space="PSUM") as psT,
            ):
                wqkv_sb = paw.tile([128, KC, 3 * C], F32R)
                nc.sync.dma_start(
                    wqkv_sb, wqkv_d[:].rearrange("(k p) n -> p k n", p=128)
                )

                for b in range(BLOC):
                    t0 = b * N
                    # LN1 + transpose to feature-major xnT [128, KC, 256]
                    xnT = pab.tile([128, KC, N], F32R, tag="xnT")
                    x_tiles = []
                    for t in range(2):
                        xt = pa.tile([128, C], F32, tag="x")
                        nc.sync.dma_start(xt, x_d[t0 + t * 128 : t0 + (t + 1) * 128, :])
                        stats = pa1.tile([128, 2, 6], F32, tag="st1")
                        xv = xt.rearrange("p (s f) -> p s f", s=2)
                        for s in range(2):
                            nc.vector.bn_stats(stats[:, s, :], xv[:, s, :])
                        mv = pa1.tile([128, 2], F32, tag="mv1")
                        nc.vector.bn_aggr(mv, stats)
                        rstd = pa1.tile([128, 1], F32, tag="rstd1")
                        nc.scalar.activation(
                            rstd, mv[:, 1:2], AF.Sqrt, bias=eps_sb, scale=1.0
                        )
                        nc.vector.reciprocal(rstd, rstd)
                        xn = pa1.tile([128, C], F32R, tag="xn")
                        nc.vector.tensor_scalar(
                            xn, xt, mv[:, 0:1], rstd, ALU.subtract, ALU.mult
                        )
                        for c in range(KC):
                            tp = psT.tile([128, 128], F32R, tag="tp")
                            nc.tensor.transpose(
                                tp, xn[:, c * 128 : (c + 1) * 128], ident
                            )
                            nc.scalar.copy(
                                xnT[:, c, t * 128 : (t + 1) * 128], tp.bitcast(F32)
                            )
                        x_tiles.append(xt)

                    # QKV. qkT chunks 0..7 = Q^T feats, 8..15 = K^T feats
                    qkT = pab.tile([128, 2 * KC, N], F32R, tag="qkT")
                    for co in range(2 * KC):
                        qp = psQ.tile([128, N], F32, tag="qp")
                        for k in range(KC):
                            nc.tensor.matmul(
                                qp,
                                wqkv_sb[:, k, co * 128 : (co + 1) * 128],
                                xnT[:, k, :],
                                start=(k == 0),
                                stop=(k == KC - 1),
                            )
                        nc.scalar.copy(qkT[:, co, :], qp)
                    # V token-major with ones column: [128, nk_chunk, h, 65]
                    v_sb = pab.tile([128, 2, H, DH + 1], F32R, tag="v")
                    for t in range(2):
                        nc.vector.tensor_copy(
                            v_sb[:, t, :, DH : DH + 1], ones_r[:, 0:H].unsqueeze(2)
                        )
                        for vc in range(2):
                            vp = psV.tile([128, 512], F32, tag="vp")
                            for k in range(KC):
                                nc.tensor.matmul(
                                    vp,
                                    xnT[:, k, t * 128 : (t + 1) * 128],
                                    wqkv_sb[:, k, 2 * C + vc * 512 : 2 * C + (vc + 1) * 512],
                                    start=(k == 0),
                                    stop=(k == KC - 1),
                                )
                            nc.scalar.copy(
                                v_sb[:, t, vc * 8 : (vc + 1) * 8, 0:DH],
                                vp.rearrange("p (h d) -> p h d", h=8),
                            )

                    # attention per head
                    oall = pab.tile([128, KC, N], F32R, tag="oall")
                    d_sb = pa1.tile([1, H, N], F32R, tag="d")
                    for h in range(H):
                        g, c2 = h // 2, h % 2
                        base = 64 * c2
                        ebh = pa.tile([128, 2, N], F32R, tag="ebh")
                        nc.sync.dma_start(
                            ebh, expb_d[:, :, h, :].rearrange("c p q -> p c q")
                        )
                        p_sb = pa.tile([128, 2, N], F32R, tag="p")
                        e_sb = pa.tile([128, 2, N], F32R, tag="e")
                        for nk in range(2):
                            sp = psS.tile([128, N], F32, tag="sp")
                            nc.tensor.matmul(
                                sp,
                                qkT[base : base + 64, KC + g, nk * 128 : (nk + 1) * 128],
                                qkT[base : base + 64, g, :],
                                start=True,
                                stop=True,
                            )
                            nc.scalar.activation(
                                e_sb[:, nk, :], sp, AF.Exp, bias=0.0, scale=0.125
                            )
                            nc.vector.tensor_mul(
                                p_sb[:, nk, :], e_sb[:, nk, :], ebh[:, nk, :]
                            )
                        op = psO.tile([128, N], F32, tag="op")
                        for nk in range(2):
                            nc.tensor.matmul(
                                op[0 : DH + 1, :],
                                v_sb[:, nk, h, :],
                                p_sb[:, nk, :],
                                start=(nk == 0),
                                stop=(nk == 1),
                            )
                        nc.scalar.copy(d_sb[0:1, h, :], op[DH : DH + 1, :])
                        bc = psBC.tile([64, N], F32, tag="bc")
                        nc.tensor.matmul(
                            bc,
                            ones_r[0:1, :],
                            d_sb[0:1, h, :],
                            start=True,
                            stop=True,
                        )
                        rd = pa1.tile([64, N], F32, tag="rd")
                        nc.vector.reciprocal(rd, bc)
                        nc.vector.tensor_mul(
                            oall[base : base + 64, g, :], op[0:DH, :], rd
                        )

                    # proj + residual -> r1_dram
                    for co in range(2):
                        wps = []
                        for kh in range(2):
                            wp = paw2.tile([128, KC // 2, 512], F32R, tag="wproj")
                            nc.sync.dma_start(
                                wp,
                                wproj_d[
                                    kh * 512 : (kh + 1) * 512,
                                    co * 512 : (co + 1) * 512,
                                ].rearrange("(k p) n -> p k n", p=128),
                            )
                            wps.append(wp)
                        for t in range(2):
                            pp = psV.tile([128, 512], F32, tag="vp")
                            for k in range(KC):
                                nc.tensor.matmul(
                                    pp,
                                    oall[:, k, t * 128 : (t + 1) * 128],
                                    wps[k // 4][:, k % 4, :],
                                    start=(k == 0),
                                    stop=(k == KC - 1),
                                )
                            st = pa.tile([128, 512], F32, tag="stg")
                            nc.vector.tensor_add(
                                st, pp, x_tiles[t][:, co * 512 : (co + 1) * 512]
                            )
                            nc.sync.dma_start(
                                r1_dram[
                                    t0 + t * 128 : t0 + (t + 1) * 128,
                                    co * 512 : (co + 1) * 512,
                                ],
                                st,
                            )

            # ---------------- Phase B: MLP + residual ----------------------
            with (
                tc.tile_pool(name="pbw", bufs=2) as pbw,
                tc.tile_pool(name="pbh", bufs=1) as pbh,
                tc.tile_pool(name="pbr", bufs=4) as pbr,
                tc.tile_pool(name="pb", bufs=2) as pb,
                tc.tile_pool(name="psF1", bufs=2, space="PSUM") as psF1,
                tc.tile_pool(name="psF2", bufs=1, space="PSUM") as psF2,
                tc.tile_pool(name="psT2", bufs=2, space="PSUM") as psT2,
            ):
                NB = 4          # token blocks of 512
                BT = TOK // NB  # 512 tokens
                for blk in range(NB):
                    t0 = blk * BT
                    xnT2 = pbh.tile([128, KC, BT], F32R, tag="xnT2")
                    r1_tiles = []
                    for t in range(4):
                        rt = pbr.tile([128, C], F32, tag="r1")
                        nc.sync.dma_start(
                            rt, r1_dram[t0 + t * 128 : t0 + (t + 1) * 128, :]
                        )
                        stats = pb.tile([128, 2, 6], F32, tag="st2")
                        rv = rt.rearrange("p (s f) -> p s f", s=2)
                        for s in range(2):
                            nc.vector.bn_stats(stats[:, s, :], rv[:, s, :])
                        mv = pb.tile([128, 2], F32, tag="mv2")
                        nc.vector.bn_aggr(mv, stats)
                        rstd = pb.tile([128, 1], F32, tag="rstd2")
                        nc.scalar.activation(
                            rstd, mv[:, 1:2], AF.Sqrt, bias=eps_sb, scale=1.0
                        )
                        nc.vector.reciprocal(rstd, rstd)
                        xn2 = pb.tile([128, C], F32R, tag="xn2")
                        nc.vector.tensor_scalar(
                            xn2, rt, mv[:, 0:1], rstd, ALU.subtract, ALU.mult
                        )
                        for c in range(KC):
                            tp = psT2.tile([128, 128], F32R, tag="tp2")
                            nc.tensor.transpose(
                                tp, xn2[:, c * 128 : (c + 1) * 128], ident
                            )
                            nc.scalar.copy(
                                xnT2[:, c, t * 128 : (t + 1) * 128], tp.bitcast(F32)
                            )
                        r1_tiles.append(rt)

                    # fc1 + gelu -> hT [128, DFF/128, BT]
                    hT = pbh.tile([128, DFF // 128, BT], F32R, tag="hT")
                    for s in range(8):      # dff slices of 512
                        wf1 = pbw.tile([128, KC, 512], F32R, tag="wf1")
                        nc.sync.dma_start(
                            wf1,
                            wfc1_d[:, s * 512 : (s + 1) * 512].rearrange(
                                "(k p) n -> p k n", p=128
                            ),
                        )
                        for dc in range(4):
                            fp = psF1.tile([128, BT], F32, tag="fp")
                            for k in range(KC):
                                nc.tensor.matmul(
                                    fp,
                                    wf1[:, k, dc * 128 : (dc + 1) * 128],
                                    xnT2[:, k, :],
                                    start=(k == 0),
                                    stop=(k == KC - 1),
                                )
                            nc.scalar.activation(
                                hT[:, s * 4 + dc, :], fp, AF.Gelu_apprx_tanh
                            )

                    # fc2 + residual -> y (wfc2 streamed in half-K chunks)
                    KF = DFF // 128
                    for co in range(2):
                        op2s = [psF2.tile([128, 512], F32, tag=f"op2_{t}", name=f"op2_{t}") for t in range(4)]
                        for kh in range(4):
                            wf2 = pbw.tile([128, KF // 4, 512], F32R, tag="wf2")
                            nc.sync.dma_start(
                                wf2,
                                wfc2_d[
                                    kh * (DFF // 4) : (kh + 1) * (DFF // 4),
                                    co * 512 : (co + 1) * 512,
                                ].rearrange("(k p) n -> p k n", p=128),
                            )
                            for t in range(4):
                                for kk in range(KF // 4):
                                    k = kh * (KF // 4) + kk
                                    nc.tensor.matmul(
                                        op2s[t],
                                        hT[:, k, t * 128 : (t + 1) * 128],
                                        wf2[:, kk, :],
                                        start=(k == 0),
                                        stop=(k == KF - 1),
                                    )
                        for t in range(4):
                            st = pb.tile([128, 512], F32, tag="stg2")
                            nc.vector.tensor_add(
                                st, op2s[t], r1_tiles[t][:, co * 512 : (co + 1) * 512]
                            )
                            nc.sync.dma_start(
                                y_d[
                                    t0 + t * 128 : t0 + (t + 1) * 128,
                                    co * 512 : (co + 1) * 512,
                                ],
                                st,
                            )

    nc.finalize()
    return nc


_NC_CACHE = {}


def _get_nc():
    if "nc" not in _NC_CACHE:
        _NC_CACHE["nc"] = build()
    return _NC_CACHE["nc"]


def kernel(**inputs):
    x = np.asarray(inputs["x"], dtype=np.float32)
    qkv_w = np.asarray(inputs["qkv_w"], dtype=np.float32)
    qkv_b = np.asarray(inputs["qkv_b"], dtype=np.float32)
    proj_w = np.asarray(inputs["proj_w"], dtype=np.float32)
    proj_b = np.asarray(inputs["proj_b"], dtype=np.float32)
    fc1_w = np.asarray(inputs["fc1_w"], dtype=np.float32)
    fc1_b = np.asarray(inputs["fc1_b"], dtype=np.float32)
    fc2_w = np.asarray(inputs["fc2_w"], dtype=np.float32)
    fc2_b = np.asarray(inputs["fc2_b"], dtype=np.float32)
    ln1_g = np.asarray(inputs["ln1_g"], dtype=np.float32)
    ln1_b = np.asarray(inputs["ln1_b"], dtype=np.float32)
    ln2_g = np.asarray(inputs["ln2_g"], dtype=np.float32)
    ln2_b = np.asarray(inputs["ln2_b"], dtype=np.float32)
    rel_pos_bias = np.asarray(inputs["rel_pos_bias"], dtype=np.float32)
    rel_pos_idx = np.asarray(inputs["rel_pos_idx"])

    assert not np.any(qkv_b) and not np.any(proj_b), "nonzero bias unsupported"
    assert not np.any(fc1_b) and not np.any(fc2_b), "nonzero bias unsupported"
    assert not np.any(ln1_b) and not np.any(ln2_b), "nonzero LN bias unsupported"

    # fold LN gammas into the following weight matrices (exact when g == 1)
    wqkv = (ln1_g[:, None] * qkv_w).astype(np.float32)
    wfc1 = (ln2_g[:, None] * fc1_w).astype(np.float32)

    # dense exp(bias) table, transposed: expb[c, p, h, q] = exp(bias[q, c*128+p, h])
    Bm = rel_pos_bias[rel_pos_idx].reshape(N, N, H)          # [q, k, h]
    T = np.exp(Bm).transpose(1, 0, 2)                        # [k, q, h]
    expb = np.ascontiguousarray(
        T.reshape(2, 128, N, H).transpose(0, 1, 3, 2)
    ).astype(np.float32)

    nc = _get_nc()
    in_maps = []
    for c in range(NCORES):
        xs = np.ascontiguousarray(
            x[c * BLOC : (c + 1) * BLOC].reshape(TOK, C)
        ).astype(np.float32)
        in_maps.append(
            dict(x=xs, wqkv=wqkv, wproj=proj_w, wfc1=wfc1, wfc2=fc2_w, expb=expb)
        )
    res = run_bass_kernel_spmd(nc, in_maps, core_ids=list(range(NCORES)))
    y = np.concatenate([res.results[c]["y"] for c in range(NCORES)], axis=0)
    return y.reshape(B, N, C).astype(np.float32)



# revision 13
# speedup vs baseline: 1.8861x; 1.8861x over previous
"""LITv1 transformer block on 8 TRN2 NeuronCores, data-parallel over batch.

v2: fp8 DoubleRow matmuls with error-feedback dual-fp8 operands.

Per-core layout (8 batches x 256 tokens):
- x/r1 residual stream resident in SBUF f32 (one 8MB buffer, updated in place)
- LN stats batched per phase (one Act Sqrt per phase -> few act-table loads)
- QKV: x-dual fp8 DoubleRow (xn hi+lo pairs in the moving slots, single-fp8
  weights broadcast stride-0 into the stationary slots) -> 0.25 cyc/k-tile
- attention internals bf16: S^T = K^T.T @ Q^T with the relative-position bias
  accumulated into PSUM via an 8*I identity matmul; exp on Act -> bf16 P;
  token-major AV with a ones-column for the softmax denominator
- proj: o-dual fp8 DoubleRow stationary, single-fp8 weights moving
- MLP: fc1 x-dual fp8 DoubleRow (xn2 hi+lo), single-fp8 W1; gelu -> fp8 h;
  fc2 w-dual fp8 DoubleRow (W2 hi+lo streamed from DRAM), h broadcast
- all transposes in bf16 via PE identity matmuls; dual-fp8 splits happen at
  the PSUM evacuation (hi = copy, lo = psum - hi via scalar_tensor_tensor)
"""
import sys

import numpy as np

sys.path.insert(0, "/opt/trn_rl_repo")

import ml_dtypes  # noqa: E402

import concourse.bass as bass  # noqa: E402
import concourse.mybir as mybir  # noqa: E402
import concourse.tile as tile  # noqa: E402
from concourse import bacc  # noqa: E402
from concourse.bass_utils import run_bass_kernel_spmd  # noqa: E402
from concourse.masks import make_identity  # noqa: E402

F32 = mybir.dt.float32
F8 = mybir.dt.float8e4
BF16 = mybir.dt.bfloat16
AF = mybir.ActivationFunctionType
ALU = mybir.AluOpType
PM = mybir.MatmulPerfMode

NP8 = ml_dtypes.float8_e4m3
NPB = ml_dtypes.bfloat16

B, N, C = 64, 256, 1024
H, DH = 16, 64
DFF = 4 * C
NCORES = 8
BLOC = B // NCORES          # 8 batches per core
TOK = BLOC * N              # 2048 tokens per core
NT = TOK // 128             # 16 token tiles of 128
KC = C // 128               # 8 contraction chunks of 128
KF = DFF // 128             # 32 dff chunks


def _pair(ap):
    """Broadcast a [128, X] AP to [128, 2, X] with stride-0 pair dim."""
    return ap.unsqueeze(1).broadcast_to([ap.shape[0], 2, ap.shape[-1]])


def build():
    nc = bacc.Bacc("TRN2")
    x_d = nc.dram_tensor("x", [TOK, C], F32, kind="ExternalInput")
    wqkv_d = nc.dram_tensor("wqkv", [128, KC, 3 * C], F8, kind="ExternalInput")
    wp_d = nc.dram_tensor("wp", [128, KC, C], F8, kind="ExternalInput")
    bt_d = nc.dram_tensor("bt", [128, 2, H, N], BF16, kind="ExternalInput")
    w1hi_d = nc.dram_tensor("w1hi", [128, KC, DFF], F8, kind="ExternalInput")
    w1lo_d = nc.dram_tensor("w1lo", [128, KC, DFF], F8, kind="ExternalInput")
    w2_d = nc.dram_tensor("w2", [128, KF, 2, C], F8, kind="ExternalInput")
    y_d = nc.dram_tensor("y", [TOK, C], F32, kind="ExternalOutput")

    with tile.TileContext(nc) as tc:
        with (
            tc.tile_pool(name="consts", bufs=1) as consts,
            tc.tile_pool(name="resid", bufs=1) as resid,
        ):
            ident_f = consts.tile([128, 128], F32)
            make_identity(nc, ident_f)
            identb = consts.tile([128, 128], BF16)
            nc.vector.tensor_copy(identb, ident_f)
            eightb = consts.tile([128, 128], BF16)
            nc.vector.tensor_scalar(eightb, ident_f, 8.0, None, ALU.mult)
            eps_sb = consts.tile([128, 1], F32)
            nc.vector.memset(eps_sb, 1e-5)

            xr = resid.tile([128, NT, C], F32)      # x then r1 then y, in place
            mvs = resid.tile([128, NT, 2], F32)     # LN1 mean/var
            rstds = resid.tile([128, NT], F32)
            mvs2 = resid.tile([128, NT, 2], F32)    # LN2 mean/var
            rstds2 = resid.tile([128, NT], F32)

            # ---------------- prologue: load x, LN1 stats --------------------
            with tc.tile_pool(name="pst", bufs=2) as pst:
                sq = pst.tile([128, NT], F32, tag="sq")
                for lo, hi_t in ((0, 4), (4, NT)):
                    for t in range(lo, hi_t):
                        nc.sync.dma_start(xr[:, t, :], x_d[t * 128:(t + 1) * 128, :])
                        st = pst.tile([128, 2, 6], F32, tag="st")
                        xv = xr[:, t, :].rearrange("p (s f) -> p s f", s=2)
                        for s in range(2):
                            nc.vector.bn_stats(st[:, s, :], xv[:, s, :])
                        nc.vector.bn_aggr(mvs[:, t, :], st)
                    nc.scalar.activation(
                        sq[:, lo:hi_t], mvs[:, lo:hi_t, 1], AF.Sqrt,
                        bias=eps_sb, scale=1.0,
                    )
                    nc.vector.reciprocal(rstds[:, lo:hi_t], sq[:, lo:hi_t])

            # ---------------- phase A: attention + proj ----------------------
            with (
                tc.tile_pool(name="paw", bufs=1) as paw,
                tc.tile_pool(name="pa1", bufs=2) as pa1,
                tc.tile_pool(name="pa2", bufs=2) as pa2,
                tc.tile_pool(name="pae", bufs=16) as pae,
                tc.tile_pool(name="psT", bufs=1, space="PSUM") as psT,
                tc.tile_pool(name="psQK", bufs=2, space="PSUM") as psQK,
                tc.tile_pool(name="psMM", bufs=2, space="PSUM") as psMM,
                tc.tile_pool(name="psS", bufs=2, space="PSUM") as psS,
                tc.tile_pool(name="psAV", bufs=1, space="PSUM") as psAV,
            ):
                wqkv_sb = paw.tile([128, KC, 3 * C], F8)
                nc.sync.dma_start(wqkv_sb, wqkv_d[:])
                wp_sb = paw.tile([128, KC, C], F8)
                nc.sync.dma_start(wp_sb, wp_d[:])
                bt_sb = paw.tile([128, 2, H, N], BF16)
                nc.sync.dma_start(bt_sb, bt_d[:])

                def tp4(pool_tag, srcs, ident):
                    """4 transposes into one PSUM bank as ONE accumulation
                    group (hardware zeroes PSUM lazily per 2KB region; separate
                    groups in one region would wipe earlier sub-tiles)."""
                    tp = psT.tile([128, 4, 128], BF16, tag=pool_tag)
                    for j, src in enumerate(srcs):
                        nc.tensor.matmul(
                            tp[:, j, :], src, ident, is_transpose=True,
                            start=(j == 0), stop=(j == 3),
                        )
                    return tp

                for b in range(BLOC):
                    # LN1 norm -> bf16, transpose, dual-fp8 evac
                    xn16 = pa1.tile([128, 2, C], BF16, tag="xn16")
                    xnT = pa1.tile([128, KC, 2, N], F8, tag="xnT")
                    for i in range(2):
                        t = 2 * b + i
                        nc.vector.tensor_scalar(
                            xn16[:, i, :], xr[:, t, :],
                            mvs[:, t, 0:1], rstds[:, t:t + 1],
                            ALU.subtract, ALU.mult,
                        )
                        for cg in range(2):
                            tp = tp4("tp", [
                                xn16[:, i, (cg * 4 + j) * 128:(cg * 4 + j + 1) * 128]
                                for j in range(4)
                            ], identb)
                            hi = xnT[:, cg * 4:(cg + 1) * 4, 0, i * 128:(i + 1) * 128]
                            nc.scalar.copy(hi, tp)
                            nc.vector.scalar_tensor_tensor(
                                xnT[:, cg * 4:(cg + 1) * 4, 1, i * 128:(i + 1) * 128],
                                tp, 1.0, hi, ALU.mult, ALU.subtract,
                            )

                    # QKV: x-dual fp8 DoubleRow
                    qkT = pa2.tile([128, 2 * KC, N], BF16, tag="qkT")
                    for fp in range(KC):       # pairs of 128-feat chunks (Q,K)
                        qp = psQK.tile([128, 2, N], F32, tag="qk")
                        for s in range(2):
                            fo = 2 * fp + s
                            for k in range(KC):
                                nc.tensor.matmul(
                                    qp[:, s, :],
                                    _pair(wqkv_sb[:, k, fo * 128:(fo + 1) * 128]),
                                    xnT[:, k, :, :],
                                    start=(s == 0 and k == 0),
                                    stop=(s == 1 and k == KC - 1),
                                    perf_mode=PM.DoubleRow,
                                )
                        nc.vector.tensor_scalar(
                            qkT[:, 2 * fp:2 * fp + 2, :], qp, 1.0 / 64, None, ALU.mult
                        )

                    # S^T + bias, exp -> bf16 P (all 16 heads)
                    es = []
                    for h in range(H):
                        sp = psS.tile([128, 2, N], F32, tag="sp")
                        r0 = (h % 2) * 64
                        for nk in range(2):
                            nc.tensor.matmul(
                                sp[:, nk, :], eightb, bt_sb[:, nk, h, :],
                                start=(nk == 0), stop=False,
                            )
                            nc.tensor.matmul(
                                sp[:, nk, :],
                                qkT[r0:r0 + 64, KC + h // 2, nk * 128:(nk + 1) * 128],
                                qkT[r0:r0 + 64, h // 2, :],
                                start=False, stop=(nk == 1),
                            )
                        e = pae.tile([128, 2, N], BF16, tag="e")
                        nc.scalar.activation(e, sp, AF.Exp, bias=0.0, scale=0.125)
                        es.append(e)

                    # V (emitted after S so PE has work while Act drains exps)
                    v_sb = pa2.tile([128, 2, H, DH + 1], BF16, tag="v")
                    nc.vector.memset(v_sb[:, :, :, DH:DH + 1], 1.0)
                    for t in range(2):
                        for vh in range(2):    # halves of V feature dim
                            vp = psMM.tile([128, 2, N], F32, tag="mm")
                            for s in range(2):
                                vq = 2 * vh + s
                                for k in range(KC):
                                    nc.tensor.matmul(
                                        vp[:, s, :],
                                        xnT[:, k, :, t * 128:(t + 1) * 128],
                                        _pair(wqkv_sb[:, k, 2 * C + vq * 256:2 * C + (vq + 1) * 256]),
                                        start=(s == 0 and k == 0),
                                        stop=(s == 1 and k == KC - 1),
                                        perf_mode=PM.DoubleRow,
                                    )
                            nc.vector.tensor_scalar(
                                v_sb[:, t, vh * 8:(vh + 1) * 8, 0:DH],
                                vp.rearrange("p s (a d) -> p (s a) d", d=DH),
                                1.0 / 64, None, ALU.mult,
                            )

                    # AV token-major + normalize -> bf16 O
                    o_sb = pa1.tile([128, 2, H, DH], BF16, tag="o")
                    rd = pa1.tile([128, 2, H], F32, tag="rd")
                    for qc in range(2):
                        for hg in range(4):
                            av = psAV.tile([128, 4, DH + 1], F32, tag="av")
                            for hh in range(4):
                                h = hg * 4 + hh
                                for nk in range(2):
                                    nc.tensor.matmul(
                                        av[:, hh, :],
                                        es[h][:, nk, qc * 128:(qc + 1) * 128],
                                        v_sb[:, nk, h, :],
                                        start=(hh == 0 and nk == 0),
                                        stop=(hh == 3 and nk == 1),
                                    )
                            nc.vector.reciprocal(
                                rd[:, qc, hg * 4:(hg + 1) * 4], av[:, :, DH]
                            )
                            for hh in range(4):
                                h = hg * 4 + hh
                                nc.vector.tensor_scalar(
                                    o_sb[:, qc, h, :], av[:, hh, 0:DH],
                                    rd[:, qc, h:h + 1], None, ALU.mult,
                                )

                    # O transpose (bf16) + dual-fp8 evac
                    oT = pa1.tile([128, KC, 2, N], F8, tag="oT")
                    for qc in range(2):
                        for cg in range(2):
                            tp = tp4("tp", [
                                o_sb[:, qc, 2 * (cg * 4 + j):2 * (cg * 4 + j) + 2, :]
                                .rearrange("p a d -> p (a d)")
                                for j in range(4)
                            ], identb)
                            hi = oT[:, cg * 4:(cg + 1) * 4, 0, qc * 128:(qc + 1) * 128]
                            nc.scalar.copy(hi, tp)
                            nc.vector.scalar_tensor_tensor(
                                oT[:, cg * 4:(cg + 1) * 4, 1, qc * 128:(qc + 1) * 128],
                                tp, 1.0, hi, ALU.mult, ALU.subtract,
                            )

                    # proj: o-dual fp8 DoubleRow + residual into xr
                    for t in range(2):
                        for ch in range(2):    # output halves of 512
                            pp = psMM.tile([128, 2, N], F32, tag="mm")
                            for s in range(2):
                                cq = 2 * ch + s
                                for k in range(KC):
                                    nc.tensor.matmul(
                                        pp[:, s, :],
                                        oT[:, k, :, t * 128:(t + 1) * 128],
                                        _pair(wp_sb[:, k, cq * 256:(cq + 1) * 256]),
                                        start=(s == 0 and k == 0),
                                        stop=(s == 1 and k == KC - 1),
                                        perf_mode=PM.DoubleRow,
                                    )
                            tt = 2 * b + t
                            nc.vector.scalar_tensor_tensor(
                                xr[:, tt, ch * 512:(ch + 1) * 512],
                                pp.rearrange("p s n -> p (s n)"), 1.0 / 64,
                                xr[:, tt, ch * 512:(ch + 1) * 512],
                                ALU.mult, ALU.add,
                            )

                    # LN2 stats for this batch's two tiles (hides the
                    # phase-boundary stats latency)
                    for i in range(2):
                        t = 2 * b + i
                        st2 = pa1.tile([128, 2, 6], F32, tag="st2a")
                        rv = xr[:, t, :].rearrange("p (s f) -> p s f", s=2)
                        for s in range(2):
                            nc.vector.bn_stats(st2[:, s, :], rv[:, s, :])
                        nc.vector.bn_aggr(mvs2[:, t, :], st2)

            # ---------------- phase B: MLP ----------------------------------
            with (
                tc.tile_pool(name="pbst", bufs=2) as pbst,
                tc.tile_pool(name="pbw", bufs=2) as pbw,
                tc.tile_pool(name="pbh", bufs=1) as pbh,
                tc.tile_pool(name="pb1", bufs=2) as pb1,
                tc.tile_pool(name="psT2", bufs=2, space="PSUM") as psT2,
                tc.tile_pool(name="psF1", bufs=3, space="PSUM") as psF1,
                tc.tile_pool(name="psF2", bufs=2, space="PSUM") as psF2,
            ):
                # LN2 stats were computed per-batch in phase A
                sq2 = pbst.tile([128, NT], F32, tag="sq2")
                nc.scalar.activation(sq2, mvs2[:, :, 1], AF.Sqrt, bias=eps_sb, scale=1.0)
                nc.vector.reciprocal(rstds2, sq2)

                NB = 2
                BT = TOK // NB          # 1024 tokens per block
                for blk in range(NB):
                    xnT2 = pbh.tile([128, KC, 2, BT], F8, tag="xnT2")
                    hT = pbh.tile([128, KF, BT], F8, tag="hT")
                    for tt in range(BT // 128):
                        t = blk * (BT // 128) + tt
                        xn2 = pb1.tile([128, C], BF16, tag="xn2")
                        nc.vector.tensor_scalar(
                            xn2, xr[:, t, :], mvs2[:, t, 0:1], rstds2[:, t:t + 1],
                            ALU.subtract, ALU.mult,
                        )
                        for cg in range(2):
                            tp = psT2.tile([128, 4, 128], BF16, tag="tp2")
                            for j in range(4):
                                nc.tensor.matmul(
                                    tp[:, j, :],
                                    xn2[:, (cg * 4 + j) * 128:(cg * 4 + j + 1) * 128],
                                    identb, is_transpose=True,
                                    start=(j == 0), stop=(j == 3),
                                )
                            hi = xnT2[:, cg * 4:(cg + 1) * 4, 0, tt * 128:(tt + 1) * 128]
                            nc.scalar.copy(hi, tp)
                            nc.vector.scalar_tensor_tensor(
                                xnT2[:, cg * 4:(cg + 1) * 4, 1, tt * 128:(tt + 1) * 128],
                                tp, 1.0, hi, ALU.mult, ALU.subtract,
                            )

                    # fc1: both-dual via 3 single-DR passes
                    # (w_hi@x_hi + w_lo@x_hi + w_hi@x_lo), gelu -> fp8 hT
                    for sl in range(8):     # dff slices of 512
                        w1hc = pbw.tile([128, KC, 512], F8, tag="w1hc")
                        nc.sync.dma_start(w1hc, w1hi_d[:, :, sl * 512:(sl + 1) * 512])
                        w1lc = pbw.tile([128, KC, 512], F8, tag="w1lc")
                        nc.sync.dma_start(w1lc, w1lo_d[:, :, sl * 512:(sl + 1) * 512])
                        for dc in range(4):
                            for tc2 in range(2):
                                fp1 = psF1.tile([128, 2, 256], F32, tag="f1")
                                passes = [(w1hc, 0), (w1lc, 0), (w1hc, 1)]
                                for th in range(2):
                                    tq = 2 * tc2 + th
                                    for pi, (wt, xi) in enumerate(passes):
                                        for kp in range(KC // 2):
                                            nc.tensor.matmul(
                                                fp1[:, th, :],
                                                wt[:, 2 * kp:2 * kp + 2,
                                                   dc * 128:(dc + 1) * 128],
                                                xnT2[:, 2 * kp:2 * kp + 2, xi,
                                                     tq * 256:(tq + 1) * 256],
                                                start=(th == 0 and pi == 0 and kp == 0),
                                                stop=(th == 1 and pi == 2
                                                      and kp == KC // 2 - 1),
                                                perf_mode=PM.DoubleRow,
                                            )
                                nc.scalar.activation(
                                    hT[:, sl * 4 + dc, tc2 * 512:(tc2 + 1) * 512],
                                    fp1.rearrange("p a b -> p (a b)"),
                                    AF.Gelu_apprx_tanh, scale=1.0 / 64,
                                )

                    # fc2: w-dual DoubleRow + residual -> y
                    for co in range(4):     # output quarters of 256
                        w2c = pbw.tile([128, KF, 2, 256], F8, tag="w2c")
                        nc.sync.dma_start(w2c, w2_d[:, :, :, co * 256:(co + 1) * 256])
                        for tcc in range(BT // 128):
                            t = blk * (BT // 128) + tcc
                            fp2 = psF2.tile([128, 256], F32, tag="f2")
                            for k in range(KF):
                                nc.tensor.matmul(
                                    fp2,
                                    _pair(hT[:, k, tcc * 128:(tcc + 1) * 128]),
                                    w2c[:, k, :, :],
                                    start=(k == 0), stop=(k == KF - 1),
                                    perf_mode=PM.DoubleRow,
                                )
                            nc.vector.scalar_tensor_tensor(
                                xr[:, t, co * 256:(co + 1) * 256], fp2, 1.0 / 64,
                                xr[:, t, co * 256:(co + 1) * 256],
                                ALU.mult, ALU.add,
                            )
                    for tcc in range(BT // 128):
                        t = blk * (BT // 128) + tcc
                        nc.sync.dma_start(
                            y_d[t * 128:(t + 1) * 128, :], xr[:, t, :]
                        )

    nc.finalize()
    return nc


_NC_CACHE = {}


def _get_nc():
    if "nc" not in _NC_CACHE:
        _NC_CACHE["nc"] = build()
    return _NC_CACHE["nc"]


def _q8(x):
    return np.clip(np.asarray(x, np.float32), -240, 240).astype(NP8)


def kernel(**inputs):
    x = np.asarray(inputs["x"], dtype=np.float32)
    qkv_w = np.asarray(inputs["qkv_w"], dtype=np.float32)
    proj_w = np.asarray(inputs["proj_w"], dtype=np.float32)
    fc1_w = np.asarray(inputs["fc1_w"], dtype=np.float32)
    fc2_w = np.asarray(inputs["fc2_w"], dtype=np.float32)
    ln1_g = np.asarray(inputs["ln1_g"], dtype=np.float32)
    ln2_g = np.asarray(inputs["ln2_g"], dtype=np.float32)
    rel_pos_bias = np.asarray(inputs["rel_pos_bias"], dtype=np.float32)
    rel_pos_idx = np.asarray(inputs["rel_pos_idx"])

    for name in ("qkv_b", "proj_b", "fc1_b", "fc2_b", "ln1_b", "ln2_b"):
        assert not np.any(np.asarray(inputs[name])), f"nonzero {name} unsupported"

    wqkv = (ln1_g[:, None] * qkv_w).reshape(KC, 128, 3 * C).transpose(1, 0, 2)
    wqkv8 = _q8(64 * wqkv)
    wp = proj_w.reshape(KC, 128, C).transpose(1, 0, 2)
    wp8 = _q8(64 * wp)
    w1 = (ln2_g[:, None] * fc1_w).reshape(KC, 128, DFF).transpose(1, 0, 2)
    w1_hi = _q8(64 * w1)
    w1_lo = _q8(64 * w1 - w1_hi.astype(np.float32))
    w2 = fc2_w.reshape(KF, 128, C).transpose(1, 0, 2)   # [128, KF, C]
    w2_hi = _q8(64 * w2)
    w2_lo = _q8(64 * w2 - w2_hi.astype(np.float32))
    w2d = np.ascontiguousarray(np.stack([w2_hi, w2_lo], axis=2))  # [128,KF,2,C]

    # device multiplies by 8*I and exp applies scale 1/8, so store B itself
    Bm = rel_pos_bias[rel_pos_idx].reshape(N, N, H)          # [q, k, h]
    bt = np.ascontiguousarray(
        Bm.transpose(1, 2, 0).reshape(2, 128, H, N).transpose(1, 0, 2, 3)
    ).astype(NPB)                                            # [128, 2(nk), H, q]

    nc = _get_nc()
    in_maps = []
    for c in range(NCORES):
        xs = np.ascontiguousarray(
            x[c * BLOC:(c + 1) * BLOC].reshape(TOK, C)
        ).astype(np.float32)
        in_maps.append(
            dict(x=xs, wqkv=wqkv8, wp=wp8, bt=bt, w1hi=w1_hi, w1lo=w1_lo, w2=w2d)
        )
    res = run_bass_kernel_spmd(nc, in_maps, core_ids=list(range(NCORES)))
    y = np.concatenate([res.results[c]["y"] for c in range(NCORES)], axis=0)
    return y.reshape(B, N, C).astype(np.float32)


# revision 18
# speedup vs baseline: 2.0680x; 1.0965x over previous
"""LITv1 transformer block on 8 TRN2 NeuronCores, data-parallel over batch.

v2: fp8 DoubleRow matmuls with error-feedback dual-fp8 operands.

Per-core layout (8 batches x 256 tokens):
- x/r1 residual stream resident in SBUF f32 (one 8MB buffer, updated in place)
- LN stats batched per phase (one Act Sqrt per phase -> few act-table loads)
- QKV: x-dual fp8 DoubleRow (xn hi+lo pairs in the moving slots, single-fp8
  weights broadcast stride-0 into the stationary slots) -> 0.25 cyc/k-tile
- attention internals bf16: S^T = K^T.T @ Q^T with the relative-position bias
  accumulated into PSUM via an 8*I identity matmul; exp on Act -> bf16 P;
  token-major AV with a ones-column for the softmax denominator
- proj: o-dual fp8 DoubleRow stationary, single-fp8 weights moving
- MLP: fc1 x-dual fp8 DoubleRow (xn2 hi+lo), single-fp8 W1; gelu -> fp8 h;
  fc2 w-dual fp8 DoubleRow (W2 hi+lo streamed from DRAM), h broadcast
- all transposes in bf16 via PE identity matmuls; dual-fp8 splits happen at
  the PSUM evacuation (hi = copy, lo = psum - hi via scalar_tensor_tensor)
"""
import sys

import numpy as np

sys.path.insert(0, "/opt/trn_rl_repo")

import ml_dtypes  # noqa: E402

import concourse.bass as bass  # noqa: E402
import concourse.mybir as mybir  # noqa: E402
import concourse.tile as tile  # noqa: E402
from concourse import bacc  # noqa: E402
from concourse.bass_utils import run_bass_kernel_spmd  # noqa: E402
from concourse.masks import make_identity  # noqa: E402

F32 = mybir.dt.float32
F8 = mybir.dt.float8e4
BF16 = mybir.dt.bfloat16
AF = mybir.ActivationFunctionType
ALU = mybir.AluOpType
PM = mybir.MatmulPerfMode

NP8 = ml_dtypes.float8_e4m3
NPB = ml_dtypes.bfloat16

B, N, C = 64, 256, 1024
H, DH = 16, 64
DFF = 4 * C
NCORES = 8
BLOC = B // NCORES          # 8 batches per core
TOK = BLOC * N              # 2048 tokens per core
NT = TOK // 128             # 16 token tiles of 128
KC = C // 128               # 8 contraction chunks of 128
KF = DFF // 128             # 32 dff chunks


def _pair(ap):
    """Broadcast a [128, X] AP to [128, 2, X] with stride-0 pair dim."""
    return ap.unsqueeze(1).broadcast_to([ap.shape[0], 2, ap.shape[-1]])


def build():
    nc = bacc.Bacc("TRN2")
    x_d = nc.dram_tensor("x", [TOK, C], F32, kind="ExternalInput")
    wqkv_d = nc.dram_tensor("wqkv", [128, KC, 3 * C], F8, kind="ExternalInput")
    wp_d = nc.dram_tensor("wp", [128, KC, C], F8, kind="ExternalInput")
    bt_d = nc.dram_tensor("bt", [128, 2, 2, H, N], F8, kind="ExternalInput")
    w1hi_d = nc.dram_tensor("w1hi", [128, KC, DFF], F8, kind="ExternalInput")
    w1lo_d = nc.dram_tensor("w1lo", [128, KC, DFF], F8, kind="ExternalInput")
    w2_d = nc.dram_tensor("w2", [128, KF, 2, C], F8, kind="ExternalInput")
    y_d = nc.dram_tensor("y", [TOK, C], F32, kind="ExternalOutput")

    with tile.TileContext(nc) as tc:
        with (
            tc.tile_pool(name="consts", bufs=1) as consts,
            tc.tile_pool(name="resid", bufs=1) as resid,
        ):
            ident_f = consts.tile([128, 128], F32)
            make_identity(nc, ident_f)
            identb = consts.tile([128, 128], BF16)
            nc.vector.tensor_copy(identb, ident_f)
            eight8 = consts.tile([128, 128], F8)
            nc.vector.tensor_scalar(eight8, ident_f, 8.0, None, ALU.mult)
            eps_sb = consts.tile([128, 1], F32)
            nc.vector.memset(eps_sb, 1e-5)

            xr = resid.tile([128, NT, C], F32)      # x then r1 then y, in place
            mvs = resid.tile([128, NT, 2], F32)     # LN1 mean/var
            rstds = resid.tile([128, NT], F32)
            mvs2 = resid.tile([128, NT, 2], F32)    # LN2 mean/var
            rstds2 = resid.tile([128, NT], F32)

            # ---------------- prologue: load x, LN1 stats --------------------
            with tc.tile_pool(name="pst", bufs=2) as pst:
                sq = pst.tile([128, NT], F32, tag="sq")
                for lo, hi_t in ((0, 4), (4, NT)):
                    for t in range(lo, hi_t):
                        nc.sync.dma_start(xr[:, t, :], x_d[t * 128:(t + 1) * 128, :])
                        st = pst.tile([128, 2, 6], F32, tag="st")
                        xv = xr[:, t, :].rearrange("p (s f) -> p s f", s=2)
                        for s in range(2):
                            nc.vector.bn_stats(st[:, s, :], xv[:, s, :])
                        nc.vector.bn_aggr(mvs[:, t, :], st)
                    nc.scalar.activation(
                        sq[:, lo:hi_t], mvs[:, lo:hi_t, 1], AF.Sqrt,
                        bias=eps_sb, scale=1.0,
                    )
                    nc.vector.reciprocal(rstds[:, lo:hi_t], sq[:, lo:hi_t])

            # ---------------- phase A: attention + proj ----------------------
            with (
                tc.tile_pool(name="paw", bufs=1) as paw,
                tc.tile_pool(name="pa1", bufs=2) as pa1,
                tc.tile_pool(name="pa2", bufs=2) as pa2,
                tc.tile_pool(name="pae", bufs=16) as pae,
                tc.tile_pool(name="psT", bufs=2, space="PSUM") as psT,
                tc.tile_pool(name="psQK", bufs=2, space="PSUM") as psQK,
                tc.tile_pool(name="psMM", bufs=1, space="PSUM") as psMM,
                tc.tile_pool(name="psS", bufs=2, space="PSUM") as psS,
                tc.tile_pool(name="psAV", bufs=1, space="PSUM") as psAV,
            ):
                wqkv_sb = paw.tile([128, KC, 3 * C], F8)
                nc.sync.dma_start(wqkv_sb, wqkv_d[:])
                wp_sb = paw.tile([128, KC, C], F8)
                nc.sync.dma_start(wp_sb, wp_d[:])
                bt_sb = paw.tile([128, 2, 2, H, N], F8)
                nc.sync.dma_start(bt_sb, bt_d[:])

                def tp4(pool_tag, srcs, ident):
                    """4 transposes into one PSUM bank as ONE accumulation
                    group (hardware zeroes PSUM lazily per 2KB region; separate
                    groups in one region would wipe earlier sub-tiles)."""
                    tp = psT.tile([128, 4, 128], BF16, tag=pool_tag)
                    for j, src in enumerate(srcs):
                        nc.tensor.matmul(
                            tp[:, j, :], src, ident, is_transpose=True,
                            start=(j == 0), stop=(j == 3),
                        )
                    return tp

                def stage1(b):
                    """LN1 norm/transpose/dual-evac + QKV(q,k) matmuls."""
                    xn16 = pa1.tile([128, 2, C], BF16, tag="xn16", name="xn16")
                    xnT = pa1.tile([128, KC, 2, N], F8, tag="xnT", name="xnT")
                    for i in range(2):
                        t = 2 * b + i
                        nc.vector.tensor_scalar(
                            xn16[:, i, :], xr[:, t, :],
                            mvs[:, t, 0:1], rstds[:, t:t + 1],
                            ALU.subtract, ALU.mult,
                        )
                        for cg in range(2):
                            tp = tp4("tp", [
                                xn16[:, i, (cg * 4 + j) * 128:(cg * 4 + j + 1) * 128]
                                for j in range(4)
                            ], identb)
                            hi = xnT[:, cg * 4:(cg + 1) * 4, 0, i * 128:(i + 1) * 128]
                            nc.scalar.copy(hi, tp)
                            nc.vector.scalar_tensor_tensor(
                                xnT[:, cg * 4:(cg + 1) * 4, 1, i * 128:(i + 1) * 128],
                                tp, 1.0, hi, ALU.mult, ALU.subtract,
                            )

                    qkT = pa2.tile([128, 2 * KC, N], BF16, tag="qkT", name="qkT")
                    for fp in range(KC):       # pairs of 128-feat chunks (Q,K)
                        qp = psQK.tile([128, 2, N], F32, tag="qk", name="qp")
                        for s in range(2):
                            fo = 2 * fp + s
                            for k in range(KC):
                                nc.tensor.matmul(
                                    qp[:, s, :],
                                    _pair(wqkv_sb[:, k, fo * 128:(fo + 1) * 128]),
                                    xnT[:, k, :, :],
                                    start=(s == 0 and k == 0),
                                    stop=(s == 1 and k == KC - 1),
                                    perf_mode=PM.DoubleRow,
                                )
                        nc.vector.tensor_scalar(
                            qkT[:, 2 * fp:2 * fp + 2, :], qp, 1.0 / 64, None, ALU.mult
                        )
                    return xnT, qkT

                def stage2(b, xnT, qkT):
                    """S^T + dual-fp8 bias + exp -> bf16 P; V matmuls."""
                    es = []
                    for h in range(H):
                        sp = psS.tile([128, 2, N], F32, tag="sp", name="sp")
                        r0 = (h % 2) * 64
                        for nk in range(2):
                            nc.tensor.matmul(
                                sp[:, nk, :], _pair(eight8),
                                bt_sb[:, :, nk, h, :],
                                start=(nk == 0), stop=False,
                                perf_mode=PM.DoubleRow,
                            )
                            nc.tensor.matmul(
                                sp[:, nk, :],
                                qkT[r0:r0 + 64, KC + h // 2, nk * 128:(nk + 1) * 128],
                                qkT[r0:r0 + 64, h // 2, :],
                                start=False, stop=(nk == 1),
                            )
                        e = pae.tile([128, 2, N], BF16, tag="e", name="e")
                        nc.scalar.activation(e, sp, AF.Exp, bias=0.0, scale=0.125)
                        es.append(e)

                    v_sb = pa2.tile([128, 2, H, DH + 1], BF16, tag="v", name="v_sb")
                    nc.vector.memset(v_sb[:, :, :, DH:DH + 1], 1.0)
                    for t in range(2):
                        for vh in range(2):    # halves of V feature dim
                            vp = psMM.tile([128, 2, N], F32, tag="mm", name="vp")
                            for s in range(2):
                                vq = 2 * vh + s
                                for k in range(KC):
                                    nc.tensor.matmul(
                                        vp[:, s, :],
                                        xnT[:, k, :, t * 128:(t + 1) * 128],
                                        _pair(wqkv_sb[:, k, 2 * C + vq * 256:2 * C + (vq + 1) * 256]),
                                        start=(s == 0 and k == 0),
                                        stop=(s == 1 and k == KC - 1),
                                        perf_mode=PM.DoubleRow,
                                    )
                            nc.vector.tensor_scalar(
                                v_sb[:, t, vh * 8:(vh + 1) * 8, 0:DH],
                                vp.rearrange("p s (a d) -> p (s a) d", d=DH),
                                1.0 / 64, None, ALU.mult,
                            )
                    return es, v_sb

                def stage3(b, es, v_sb):
                    """AV + normalize, O transpose, proj + residual, LN2 stats."""
                    o_sb = pa1.tile([128, 2, H, DH], BF16, tag="o", name="o_sb")
                    rd = pa1.tile([128, 2, H], F32, tag="rd", name="rd")
                    oT = pa1.tile([128, KC, 2, N], F8, tag="oT", name="oT")
                    for qc in range(2):
                        for hg in range(4):
                            av = psAV.tile([128, 4, DH + 1], F32, tag="av", name="av")
                            for hh in range(4):
                                h = hg * 4 + hh
                                for nk in range(2):
                                    nc.tensor.matmul(
                                        av[:, hh, :],
                                        es[h][:, nk, qc * 128:(qc + 1) * 128],
                                        v_sb[:, nk, h, :],
                                        start=(hh == 0 and nk == 0),
                                        stop=(hh == 3 and nk == 1),
                                    )
                            nc.vector.reciprocal(
                                rd[:, qc, hg * 4:(hg + 1) * 4], av[:, :, DH]
                            )
                            for hh in range(4):
                                h = hg * 4 + hh
                                nc.vector.tensor_scalar(
                                    o_sb[:, qc, h, :], av[:, hh, 0:DH],
                                    rd[:, qc, h:h + 1], None, ALU.mult,
                                )
                        for cg in range(2):
                            tp = tp4("tp", [
                                o_sb[:, qc, 2 * (cg * 4 + j):2 * (cg * 4 + j) + 2, :]
                                .rearrange("p a d -> p (a d)")
                                for j in range(4)
                            ], identb)
                            hi = oT[:, cg * 4:(cg + 1) * 4, 0, qc * 128:(qc + 1) * 128]
                            nc.scalar.copy(hi, tp)
                            nc.vector.scalar_tensor_tensor(
                                oT[:, cg * 4:(cg + 1) * 4, 1, qc * 128:(qc + 1) * 128],
                                tp, 1.0, hi, ALU.mult, ALU.subtract,
                            )

                    for t in range(2):
                        for ch in range(2):    # output halves of 512
                            pp = psMM.tile([128, 2, N], F32, tag="mm", name="pp")
                            for s in range(2):
                                cq = 2 * ch + s
                                for k in range(KC):
                                    nc.tensor.matmul(
                                        pp[:, s, :],
                                        oT[:, k, :, t * 128:(t + 1) * 128],
                                        _pair(wp_sb[:, k, cq * 256:(cq + 1) * 256]),
                                        start=(s == 0 and k == 0),
                                        stop=(s == 1 and k == KC - 1),
                                        perf_mode=PM.DoubleRow,
                                    )
                            tt = 2 * b + t
                            nc.vector.scalar_tensor_tensor(
                                xr[:, tt, ch * 512:(ch + 1) * 512],
                                pp.rearrange("p s n -> p (s n)"), 1.0 / 64,
                                xr[:, tt, ch * 512:(ch + 1) * 512],
                                ALU.mult, ALU.add,
                            )
                        tt = 2 * b + t
                        st2 = pa1.tile([128, 2, 6], F32, tag="st2a", name="st2")
                        rv = xr[:, tt, :].rearrange("p (s f) -> p s f", s=2)
                        for s in range(2):
                            nc.vector.bn_stats(st2[:, s, :], rv[:, s, :])
                        nc.vector.bn_aggr(mvs2[:, tt, :], st2)

                # software pipeline: stage3(b-1) slots between stage1(b)
                # and stage2(b) so PE never waits on Act exps / DVE evacs
                carry = None
                for b in range(BLOC):
                    xnT, qkT = stage1(b)
                    if carry is not None:
                        stage3(b - 1, *carry)
                    carry = stage2(b, xnT, qkT)
                stage3(BLOC - 1, *carry)

            # ---------------- phase B: MLP ----------------------------------
            with (
                tc.tile_pool(name="pbst", bufs=2) as pbst,
                tc.tile_pool(name="pbw", bufs=2) as pbw,
                tc.tile_pool(name="pbh", bufs=1) as pbh,
                tc.tile_pool(name="pb1", bufs=2) as pb1,
                tc.tile_pool(name="psT2", bufs=2, space="PSUM") as psT2,
                tc.tile_pool(name="psF1", bufs=3, space="PSUM") as psF1,
                tc.tile_pool(name="psF2", bufs=2, space="PSUM") as psF2,
            ):
                # LN2 stats were computed per-batch in phase A
                sq2 = pbst.tile([128, NT], F32, tag="sq2")
                nc.scalar.activation(sq2, mvs2[:, :, 1], AF.Sqrt, bias=eps_sb, scale=1.0)
                nc.vector.reciprocal(rstds2, sq2)

                NB = 2
                BT = TOK // NB          # 1024 tokens per block
                for blk in range(NB):
                    xnT2 = pbh.tile([128, KC, 2, BT], F8, tag="xnT2")
                    hT = pbh.tile([128, KF, BT], F8, tag="hT")
                    for tt in range(BT // 128):
                        t = blk * (BT // 128) + tt
                        xn2 = pb1.tile([128, C], BF16, tag="xn2")
                        nc.vector.tensor_scalar(
                            xn2, xr[:, t, :], mvs2[:, t, 0:1], rstds2[:, t:t + 1],
                            ALU.subtract, ALU.mult,
                        )
                        for cg in range(2):
                            tp = psT2.tile([128, 4, 128], BF16, tag="tp2")
                            for j in range(4):
                                nc.tensor.matmul(
                                    tp[:, j, :],
                                    xn2[:, (cg * 4 + j) * 128:(cg * 4 + j + 1) * 128],
                                    identb, is_transpose=True,
                                    start=(j == 0), stop=(j == 3),
                                )
                            hi = xnT2[:, cg * 4:(cg + 1) * 4, 0, tt * 128:(tt + 1) * 128]
                            nc.scalar.copy(hi, tp)
                            nc.vector.scalar_tensor_tensor(
                                xnT2[:, cg * 4:(cg + 1) * 4, 1, tt * 128:(tt + 1) * 128],
                                tp, 1.0, hi, ALU.mult, ALU.subtract,
                            )

                    # fc1: both-dual via 3 single-DR passes
                    # (w_hi@x_hi + w_lo@x_hi + w_hi@x_lo), gelu -> fp8 hT
                    for sl in range(8):     # dff slices of 512
                        w1hc = pbw.tile([128, KC, 512], F8, tag="w1hc")
                        nc.sync.dma_start(w1hc, w1hi_d[:, :, sl * 512:(sl + 1) * 512])
                        w1lc = pbw.tile([128, KC, 512], F8, tag="w1lc")
                        nc.sync.dma_start(w1lc, w1lo_d[:, :, sl * 512:(sl + 1) * 512])
                        for dc in range(4):
                            for tc2 in range(2):
                                fp1 = psF1.tile([128, 2, 256], F32, tag="f1")
                                passes = [(w1hc, 0), (w1lc, 0), (w1hc, 1)]
                                for th in range(2):
                                    tq = 2 * tc2 + th
                                    for pi, (wt, xi) in enumerate(passes):
                                        for kp in range(KC // 2):
                                            nc.tensor.matmul(
                                                fp1[:, th, :],
                                                wt[:, 2 * kp:2 * kp + 2,
                                                   dc * 128:(dc + 1) * 128],
                                                xnT2[:, 2 * kp:2 * kp + 2, xi,
                                                     tq * 256:(tq + 1) * 256],
                                                start=(th == 0 and pi == 0 and kp == 0),
                                                stop=(th == 1 and pi == 2
                                                      and kp == KC // 2 - 1),
                                                perf_mode=PM.DoubleRow,
                                            )
                                nc.scalar.activation(
                                    hT[:, sl * 4 + dc, tc2 * 512:(tc2 + 1) * 512],
                                    fp1.rearrange("p a b -> p (a b)"),
                                    AF.Gelu_apprx_tanh, scale=1.0 / 64,
                                )

                    # fc2: w-dual DoubleRow + residual -> y
                    for co in range(4):     # output quarters of 256
                        w2c = pbw.tile([128, KF, 2, 256], F8, tag="w2c")
                        nc.sync.dma_start(w2c, w2_d[:, :, :, co * 256:(co + 1) * 256])
                        for tcc in range(BT // 128):
                            t = blk * (BT // 128) + tcc
                            fp2 = psF2.tile([128, 256], F32, tag="f2")
                            for k in range(KF):
                                nc.tensor.matmul(
                                    fp2,
                                    _pair(hT[:, k, tcc * 128:(tcc + 1) * 128]),
                                    w2c[:, k, :, :],
                                    start=(k == 0), stop=(k == KF - 1),
                                    perf_mode=PM.DoubleRow,
                                )
                            nc.vector.scalar_tensor_tensor(
                                xr[:, t, co * 256:(co + 1) * 256], fp2, 1.0 / 64,
                                xr[:, t, co * 256:(co + 1) * 256],
                                ALU.mult, ALU.add,
                            )
                    for tcc in range(BT // 128):
                        t = blk * (BT // 128) + tcc
                        nc.sync.dma_start(
                            y_d[t * 128:(t + 1) * 128, :], xr[:, t, :]
                        )

    nc.finalize()
    return nc


_NC_CACHE = {}


def _get_nc():
    if "nc" not in _NC_CACHE:
        _NC_CACHE["nc"] = build()
    return _NC_CACHE["nc"]


def _q8(x):
    return np.clip(np.asarray(x, np.float32), -240, 240).astype(NP8)


def kernel(**inputs):
    x = np.asarray(inputs["x"], dtype=np.float32)
    qkv_w = np.asarray(inputs["qkv_w"], dtype=np.float32)
    proj_w = np.asarray(inputs["proj_w"], dtype=np.float32)
    fc1_w = np.asarray(inputs["fc1_w"], dtype=np.float32)
    fc2_w = np.asarray(inputs["fc2_w"], dtype=np.float32)
    ln1_g = np.asarray(inputs["ln1_g"], dtype=np.float32)
    ln2_g = np.asarray(inputs["ln2_g"], dtype=np.float32)
    rel_pos_bias = np.asarray(inputs["rel_pos_bias"], dtype=np.float32)
    rel_pos_idx = np.asarray(inputs["rel_pos_idx"])

    for name in ("qkv_b", "proj_b", "fc1_b", "fc2_b", "ln1_b", "ln2_b"):
        assert not np.any(np.asarray(inputs[name])), f"nonzero {name} unsupported"

    wqkv = (ln1_g[:, None] * qkv_w).reshape(KC, 128, 3 * C).transpose(1, 0, 2)
    wqkv8 = _q8(64 * wqkv)
    wp = proj_w.reshape(KC, 128, C).transpose(1, 0, 2)
    wp8 = _q8(64 * wp)
    w1 = (ln2_g[:, None] * fc1_w).reshape(KC, 128, DFF).transpose(1, 0, 2)
    w1_hi = _q8(64 * w1)
    w1_lo = _q8(64 * w1 - w1_hi.astype(np.float32))
    w2 = fc2_w.reshape(KF, 128, C).transpose(1, 0, 2)   # [128, KF, C]
    w2_hi = _q8(64 * w2)
    w2_lo = _q8(64 * w2 - w2_hi.astype(np.float32))
    w2d = np.ascontiguousarray(np.stack([w2_hi, w2_lo], axis=2))  # [128,KF,2,C]

    # device multiplies by 8*I (DoubleRow pair) and exp applies scale 1/8,
    # so store a dual-fp8 split of B itself: [128, 2(hi/lo), 2(nk), H, q]
    Bm = rel_pos_bias[rel_pos_idx].reshape(N, N, H)          # [q, k, h]
    btf = np.ascontiguousarray(
        Bm.transpose(1, 2, 0).reshape(2, 128, H, N).transpose(1, 0, 2, 3)
    ).astype(np.float32)                                     # [128, 2(nk), H, q]
    bt_hi = _q8(btf)
    bt_lo = _q8(btf - bt_hi.astype(np.float32))
    bt = np.ascontiguousarray(np.stack([bt_hi, bt_lo], axis=1))

    nc = _get_nc()
    in_maps = []
    for c in range(NCORES):
        xs = np.ascontiguousarray(
            x[c * BLOC:(c + 1) * BLOC].reshape(TOK, C)
        ).astype(np.float32)
        in_maps.append(
            dict(x=xs, wqkv=wqkv8, wp=wp8, bt=bt, w1hi=w1_hi, w1lo=w1_lo, w2=w2d)
        )
    res = run_bass_kernel_spmd(nc, in_maps, core_ids=list(range(NCORES)))
    y = np.concatenate([res.results[c]["y"] for c in range(NCORES)], axis=0)
    return y.reshape(B, N, C).astype(np.float32)


# revision 23
# speedup vs baseline: 2.3437x; 1.1333x over previous
"""LITv1 transformer block on 8 TRN2 NeuronCores, data-parallel over batch.

v2: fp8 DoubleRow matmuls with error-feedback dual-fp8 operands.

Per-core layout (8 batches x 256 tokens):
- x/r1 residual stream resident in SBUF f32 (one 8MB buffer, updated in place)
- LN stats batched per phase (one Act Sqrt per phase -> few act-table loads)
- QKV: x-dual fp8 DoubleRow (xn hi+lo pairs in the moving slots, single-fp8
  weights broadcast stride-0 into the stationary slots) -> 0.25 cyc/k-tile
- attention internals bf16: S^T = K^T.T @ Q^T with the relative-position bias
  accumulated into PSUM via an 8*I identity matmul; exp on Act -> bf16 P;
  token-major AV with a ones-column for the softmax denominator
- proj: o-dual fp8 DoubleRow stationary, single-fp8 weights moving
- MLP: fc1 x-dual fp8 DoubleRow (xn2 hi+lo), single-fp8 W1; gelu -> fp8 h;
  fc2 w-dual fp8 DoubleRow (W2 hi+lo streamed from DRAM), h broadcast
- all transposes in bf16 via PE identity matmuls; dual-fp8 splits happen at
  the PSUM evacuation (hi = copy, lo = psum - hi via scalar_tensor_tensor)
"""
import sys

import numpy as np

sys.path.insert(0, "/opt/trn_rl_repo")

import ml_dtypes  # noqa: E402

import concourse.bass as bass  # noqa: E402
import concourse.mybir as mybir  # noqa: E402
import concourse.tile as tile  # noqa: E402
from concourse import bacc  # noqa: E402
from concourse.bass_utils import run_bass_kernel_spmd  # noqa: E402
from concourse.masks import make_identity  # noqa: E402

F32 = mybir.dt.float32
F8 = mybir.dt.float8e4
BF16 = mybir.dt.bfloat16
AF = mybir.ActivationFunctionType
ALU = mybir.AluOpType
PM = mybir.MatmulPerfMode

NP8 = ml_dtypes.float8_e4m3
NPB = ml_dtypes.bfloat16

B, N, C = 64, 256, 1024
H, DH = 16, 64
DFF = 4 * C
NCORES = 8
BLOC = B // NCORES          # 8 batches per core
TOK = BLOC * N              # 2048 tokens per core
NT = TOK // 128             # 16 token tiles of 128
KC = C // 128               # 8 contraction chunks of 128
KF = DFF // 128             # 32 dff chunks


def _pair(ap):
    """Broadcast a [128, X] AP to [128, 2, X] with stride-0 pair dim."""
    return ap.unsqueeze(1).broadcast_to([ap.shape[0], 2, ap.shape[-1]])


def build():
    nc = bacc.Bacc("TRN2")
    x_d = nc.dram_tensor("x", [TOK, C], F32, kind="ExternalInput")
    wqkv_d = nc.dram_tensor("wqkv", [128, KC, 3 * C], F8, kind="ExternalInput")
    wp_d = nc.dram_tensor("wp", [128, KC, C], F8, kind="ExternalInput")
    bt_d = nc.dram_tensor("bt", [128, 2, 2, H, N], F8, kind="ExternalInput")
    w1hi_d = nc.dram_tensor("w1hi", [128, KC, DFF], F8, kind="ExternalInput")
    w1lo_d = nc.dram_tensor("w1lo", [128, KC, DFF], F8, kind="ExternalInput")
    w2_d = nc.dram_tensor("w2", [128, KF, 2, C], F8, kind="ExternalInput")
    y_d = nc.dram_tensor("y", [TOK, C], F32, kind="ExternalOutput")

    with tile.TileContext(nc) as tc:
        with (
            tc.tile_pool(name="consts", bufs=1) as consts,
            tc.tile_pool(name="resid", bufs=1) as resid,
        ):
            ident_f = consts.tile([128, 128], F32)
            make_identity(nc, ident_f)
            identb = consts.tile([128, 128], BF16)
            nc.vector.tensor_copy(identb, ident_f)
            eight8 = consts.tile([128, 128], F8)
            nc.vector.tensor_scalar(eight8, ident_f, 8.0, None, ALU.mult)
            eps_sb = consts.tile([128, 1], F32)
            nc.vector.memset(eps_sb, 1e-5)

            xr = resid.tile([128, NT, C], F32)      # x then r1 then y, in place
            mvs = resid.tile([128, NT, 2], F32)     # LN1 mean/var
            rstds = resid.tile([128, NT], F32)
            mvs2 = resid.tile([128, NT, 2], F32)    # LN2 mean/var
            rstds2 = resid.tile([128, NT], F32)

            # ---------------- prologue: load x, LN1 stats --------------------
            pst_ctx = tc.tile_pool(name="pst", bufs=2)
            pst = pst_ctx.__enter__()
            sq = pst.tile([128, NT], F32, tag="sq")

            def ln1_stats(lo, hi_t):
                for t in range(lo, hi_t):
                    st = pst.tile([128, 2, 6], F32, tag="st", name="st")
                    xv = xr[:, t, :].rearrange("p (s f) -> p s f", s=2)
                    for s in range(2):
                        nc.vector.bn_stats(st[:, s, :], xv[:, s, :])
                    nc.vector.bn_aggr(mvs[:, t, :], st)
                nc.scalar.activation(
                    sq[:, lo:hi_t], mvs[:, lo:hi_t, 1], AF.Sqrt,
                    bias=eps_sb, scale=1.0,
                )
                nc.vector.reciprocal(rstds[:, lo:hi_t], sq[:, lo:hi_t])

            # first 4 x tiles unblock batch 0 quickly
            for t in range(4):
                nc.sync.dma_start(xr[:, t, :], x_d[t * 128:(t + 1) * 128, :])
            ln1_stats(0, 4)

            # ---------------- phase A: attention + proj ----------------------
            with (
                tc.tile_pool(name="paw", bufs=1) as paw,
                tc.tile_pool(name="pa1", bufs=2) as pa1,
                tc.tile_pool(name="pa2", bufs=2) as pa2,
                tc.tile_pool(name="pae", bufs=16) as pae,
                tc.tile_pool(name="psT", bufs=2, space="PSUM") as psT,
                tc.tile_pool(name="psQK", bufs=1, space="PSUM") as psQK,
                tc.tile_pool(name="psMM", bufs=1, space="PSUM") as psMM,
                tc.tile_pool(name="psS", bufs=2, space="PSUM") as psS,
                tc.tile_pool(name="psAV", bufs=2, space="PSUM") as psAV,
            ):
                # DMA order: wqkv before remaining x tiles so batch 0's QKV
                # isn't blocked; bt before wp (bias needed earlier than proj)
                wqkv_sb = paw.tile([128, KC, 3 * C], F8)
                nc.sync.dma_start(wqkv_sb, wqkv_d[:])
                bt_sb = paw.tile([128, 2, 2, H, N], F8)
                nc.sync.dma_start(bt_sb, bt_d[:])
                wp_sb = paw.tile([128, KC, C], F8)
                nc.sync.dma_start(wp_sb, wp_d[:])
                for t in range(4, NT):
                    nc.sync.dma_start(xr[:, t, :], x_d[t * 128:(t + 1) * 128, :])

                def tp4(pool_tag, srcs, ident):
                    """4 transposes into one PSUM bank as ONE accumulation
                    group (hardware zeroes PSUM lazily per 2KB region; separate
                    groups in one region would wipe earlier sub-tiles)."""
                    tp = psT.tile([128, 4, 128], BF16, tag=pool_tag)
                    for j, src in enumerate(srcs):
                        nc.tensor.matmul(
                            tp[:, j, :], src, ident, is_transpose=True,
                            start=(j == 0), stop=(j == 3),
                        )
                    return tp

                def stage1(b):
                    """LN1 norm/transpose/dual-evac + QKV(q,k) matmuls."""
                    xn16 = pa1.tile([128, 2, C], BF16, tag="xn16", name="xn16")
                    xnT = pa1.tile([128, KC, 2, N], F8, tag="xnT", name="xnT")
                    for i in range(2):
                        t = 2 * b + i
                        nc.vector.tensor_scalar(
                            xn16[:, i, :], xr[:, t, :],
                            mvs[:, t, 0:1], rstds[:, t:t + 1],
                            ALU.subtract, ALU.mult,
                        )
                        for cg in range(2):
                            tp = tp4("tp", [
                                xn16[:, i, (cg * 4 + j) * 128:(cg * 4 + j + 1) * 128]
                                for j in range(4)
                            ], identb)
                            hi = xnT[:, cg * 4:(cg + 1) * 4, 0, i * 128:(i + 1) * 128]
                            nc.scalar.copy(hi, tp)
                            nc.vector.scalar_tensor_tensor(
                                xnT[:, cg * 4:(cg + 1) * 4, 1, i * 128:(i + 1) * 128],
                                tp, 1.0, hi, ALU.mult, ALU.subtract,
                            )

                    qkT = pa2.tile([128, 2 * KC, N], BF16, tag="qkT", name="qkT")
                    for fp in range(KC):       # pairs of 128-feat chunks (Q,K)
                        qp = psQK.tile([128, 2, N], F32, tag="qk", name="qp")
                        for s in range(2):
                            fo = 2 * fp + s
                            for k in range(KC):
                                nc.tensor.matmul(
                                    qp[:, s, :],
                                    _pair(wqkv_sb[:, k, fo * 128:(fo + 1) * 128]),
                                    xnT[:, k, :, :],
                                    start=(s == 0 and k == 0),
                                    stop=(s == 1 and k == KC - 1),
                                    perf_mode=PM.DoubleRow,
                                )
                        nc.vector.tensor_scalar(
                            qkT[:, 2 * fp:2 * fp + 2, :], qp, 1.0 / 64, None, ALU.mult
                        )
                    return xnT, qkT

                def stage2(b, xnT, qkT):
                    """S^T + dual-fp8 bias + exp -> bf16 P, with the V matmuls
                    interleaved so PE has work while Act drains the exps."""
                    es = []
                    v_sb = pa2.tile([128, 2, H, DH + 1], BF16, tag="v", name="v_sb")
                    nc.vector.memset(v_sb[:, :, :, DH:DH + 1], 1.0)

                    def s_head(h):
                        sp = psS.tile([128, 2, N], F32, tag="sp", name="sp")
                        r0 = (h % 2) * 64
                        for nk in range(2):
                            nc.tensor.matmul(
                                sp[:, nk, :], _pair(eight8),
                                bt_sb[:, :, nk, h, :],
                                start=(nk == 0), stop=False,
                                perf_mode=PM.DoubleRow,
                            )
                            nc.tensor.matmul(
                                sp[:, nk, :],
                                qkT[r0:r0 + 64, KC + h // 2, nk * 128:(nk + 1) * 128],
                                qkT[r0:r0 + 64, h // 2, :],
                                start=False, stop=(nk == 1),
                            )
                        e = pae.tile([128, 2, N], BF16, tag="e", name="e")
                        nc.scalar.activation(e, sp, AF.Exp, bias=0.0, scale=0.125)
                        es.append(e)

                    def v_half(t):
                        for vh in range(2):    # halves of V feature dim
                            vp = psMM.tile([128, 2, N], F32, tag="mm", name="vp")
                            for s in range(2):
                                vq = 2 * vh + s
                                for k in range(KC):
                                    nc.tensor.matmul(
                                        vp[:, s, :],
                                        xnT[:, k, :, t * 128:(t + 1) * 128],
                                        _pair(wqkv_sb[:, k, 2 * C + vq * 256:2 * C + (vq + 1) * 256]),
                                        start=(s == 0 and k == 0),
                                        stop=(s == 1 and k == KC - 1),
                                        perf_mode=PM.DoubleRow,
                                    )
                            nc.vector.tensor_scalar(
                                v_sb[:, t, vh * 8:(vh + 1) * 8, 0:DH],
                                vp.rearrange("p s (a d) -> p (s a) d", d=DH),
                                1.0 / 64, None, ALU.mult,
                            )

                    for h in range(H // 2):
                        s_head(h)
                    v_half(0)
                    for h in range(H // 2, H):
                        s_head(h)
                    v_half(1)
                    return es, v_sb

                def stage3(b, es, v_sb):
                    """AV + normalize, O transpose, proj + residual, LN2 stats."""
                    o_sb = pa1.tile([128, 2, H, DH], BF16, tag="o", name="o_sb")
                    rd = pa1.tile([128, 2, H], F32, tag="rd", name="rd")
                    oT = pa1.tile([128, KC, 2, N], F8, tag="oT", name="oT")
                    for qc in range(2):
                        for hg in range(4):
                            av = psAV.tile([128, 4, DH + 1], F32, tag="av", name="av")
                            for hh in range(4):
                                h = hg * 4 + hh
                                for nk in range(2):
                                    nc.tensor.matmul(
                                        av[:, hh, :],
                                        es[h][:, nk, qc * 128:(qc + 1) * 128],
                                        v_sb[:, nk, h, :],
                                        start=(hh == 0 and nk == 0),
                                        stop=(hh == 3 and nk == 1),
                                    )
                            nc.vector.reciprocal(
                                rd[:, qc, hg * 4:(hg + 1) * 4], av[:, :, DH]
                            )
                            for hh in range(4):
                                h = hg * 4 + hh
                                nc.vector.tensor_scalar(
                                    o_sb[:, qc, h, :], av[:, hh, 0:DH],
                                    rd[:, qc, h:h + 1], None, ALU.mult,
                                )
                        for cg in range(2):
                            tp = tp4("tp", [
                                o_sb[:, qc, 2 * (cg * 4 + j):2 * (cg * 4 + j) + 2, :]
                                .rearrange("p a d -> p (a d)")
                                for j in range(4)
                            ], identb)
                            hi = oT[:, cg * 4:(cg + 1) * 4, 0, qc * 128:(qc + 1) * 128]
                            nc.scalar.copy(hi, tp)
                            nc.vector.scalar_tensor_tensor(
                                oT[:, cg * 4:(cg + 1) * 4, 1, qc * 128:(qc + 1) * 128],
                                tp, 1.0, hi, ALU.mult, ALU.subtract,
                            )

                    for t in range(2):
                        for ch in range(2):    # output halves of 512
                            pp = psMM.tile([128, 2, N], F32, tag="mm", name="pp")
                            for s in range(2):
                                cq = 2 * ch + s
                                for k in range(KC):
                                    nc.tensor.matmul(
                                        pp[:, s, :],
                                        oT[:, k, :, t * 128:(t + 1) * 128],
                                        _pair(wp_sb[:, k, cq * 256:(cq + 1) * 256]),
                                        start=(s == 0 and k == 0),
                                        stop=(s == 1 and k == KC - 1),
                                        perf_mode=PM.DoubleRow,
                                    )
                            tt = 2 * b + t
                            nc.vector.scalar_tensor_tensor(
                                xr[:, tt, ch * 512:(ch + 1) * 512],
                                pp.rearrange("p s n -> p (s n)"), 1.0 / 64,
                                xr[:, tt, ch * 512:(ch + 1) * 512],
                                ALU.mult, ALU.add,
                            )
                        tt = 2 * b + t
                        st2 = pa1.tile([128, 2, 6], F32, tag="st2a", name="st2")
                        rv = xr[:, tt, :].rearrange("p (s f) -> p s f", s=2)
                        for s in range(2):
                            nc.vector.bn_stats(st2[:, s, :], rv[:, s, :])
                        nc.vector.bn_aggr(mvs2[:, tt, :], st2)

                # software pipeline: stage3(b-1) slots between stage1(b)
                # and stage2(b) so PE never waits on Act exps / DVE evacs
                carry = None
                for b in range(BLOC):
                    xnT, qkT = stage1(b)
                    if b == 0:
                        ln1_stats(4, NT)
                    if carry is not None:
                        stage3(b - 1, *carry)
                    carry = stage2(b, xnT, qkT)
                stage3(BLOC - 1, *carry)

            # ---------------- phase B: MLP ----------------------------------
            with (
                tc.tile_pool(name="pbst", bufs=2) as pbst,
                tc.tile_pool(name="pbw", bufs=2) as pbw,
                tc.tile_pool(name="pbh", bufs=1) as pbh,
                tc.tile_pool(name="pb1", bufs=2) as pb1,
                tc.tile_pool(name="psT2", bufs=2, space="PSUM") as psT2,
                tc.tile_pool(name="psF1", bufs=3, space="PSUM") as psF1,
                tc.tile_pool(name="psF2", bufs=2, space="PSUM") as psF2,
            ):
                # LN2 stats were computed per-batch in phase A
                sq2 = pbst.tile([128, NT], F32, tag="sq2")
                nc.scalar.activation(sq2, mvs2[:, :, 1], AF.Sqrt, bias=eps_sb, scale=1.0)
                nc.vector.reciprocal(rstds2, sq2)

                NB = 2
                BT = TOK // NB          # 1024 tokens per block
                for blk in range(NB):
                    xnT2 = pbh.tile([128, KC, 2, BT], F8, tag="xnT2")
                    hT = pbh.tile([128, KF, BT], F8, tag="hT")
                    for tt in range(BT // 128):
                        t = blk * (BT // 128) + tt
                        xn2 = pb1.tile([128, C], BF16, tag="xn2")
                        nc.vector.tensor_scalar(
                            xn2, xr[:, t, :], mvs2[:, t, 0:1], rstds2[:, t:t + 1],
                            ALU.subtract, ALU.mult,
                        )
                        for cg in range(2):
                            tp = psT2.tile([128, 4, 128], BF16, tag="tp2")
                            for j in range(4):
                                nc.tensor.matmul(
                                    tp[:, j, :],
                                    xn2[:, (cg * 4 + j) * 128:(cg * 4 + j + 1) * 128],
                                    identb, is_transpose=True,
                                    start=(j == 0), stop=(j == 3),
                                )
                            hi = xnT2[:, cg * 4:(cg + 1) * 4, 0, tt * 128:(tt + 1) * 128]
                            nc.scalar.copy(hi, tp)
                            nc.vector.scalar_tensor_tensor(
                                xnT2[:, cg * 4:(cg + 1) * 4, 1, tt * 128:(tt + 1) * 128],
                                tp, 1.0, hi, ALU.mult, ALU.subtract,
                            )

                    # fc1: both-dual via 3 single-DR passes
                    # (w_hi@x_hi + w_lo@x_hi + w_hi@x_lo), gelu -> fp8 hT
                    for sl in range(8):     # dff slices of 512
                        w1hc = pbw.tile([128, KC, 512], F8, tag="w1hc")
                        nc.sync.dma_start(w1hc, w1hi_d[:, :, sl * 512:(sl + 1) * 512])
                        w1lc = pbw.tile([128, KC, 512], F8, tag="w1lc")
                        nc.sync.dma_start(w1lc, w1lo_d[:, :, sl * 512:(sl + 1) * 512])
                        for dc in range(4):
                            for tc2 in range(2):
                                fp1 = psF1.tile([128, 2, 256], F32, tag="f1")
                                passes = [(w1hc, 0), (w1lc, 0), (w1hc, 1)]
                                for th in range(2):
                                    tq = 2 * tc2 + th
                                    for pi, (wt, xi) in enumerate(passes):
                                        for kp in range(KC // 2):
                                            nc.tensor.matmul(
                                                fp1[:, th, :],
                                                wt[:, 2 * kp:2 * kp + 2,
                                                   dc * 128:(dc + 1) * 128],
                                                xnT2[:, 2 * kp:2 * kp + 2, xi,
                                                     tq * 256:(tq + 1) * 256],
                                                start=(th == 0 and pi == 0 and kp == 0),
                                                stop=(th == 1 and pi == 2
                                                      and kp == KC // 2 - 1),
                                                perf_mode=PM.DoubleRow,
                                            )
                                nc.scalar.activation(
                                    hT[:, sl * 4 + dc, tc2 * 512:(tc2 + 1) * 512],
                                    fp1.rearrange("p a b -> p (a b)"),
                                    AF.Gelu_apprx_tanh, scale=1.0 / 64,
                                )

                    # fc2: w-dual DoubleRow + residual -> y
                    for co in range(4):     # output quarters of 256
                        w2c = pbw.tile([128, KF, 2, 256], F8, tag="w2c")
                        nc.sync.dma_start(w2c, w2_d[:, :, :, co * 256:(co + 1) * 256])
                        for tcc in range(BT // 128):
                            t = blk * (BT // 128) + tcc
                            fp2 = psF2.tile([128, 256], F32, tag="f2")
                            for k in range(KF):
                                nc.tensor.matmul(
                                    fp2,
                                    _pair(hT[:, k, tcc * 128:(tcc + 1) * 128]),
                                    w2c[:, k, :, :],
                                    start=(k == 0), stop=(k == KF - 1),
                                    perf_mode=PM.DoubleRow,
                                )
                            nc.vector.scalar_tensor_tensor(
                                xr[:, t, co * 256:(co + 1) * 256], fp2, 1.0 / 64,
                                xr[:, t, co * 256:(co + 1) * 256],
                                ALU.mult, ALU.add,
                            )
                    for tcc in range(BT // 128):
                        t = blk * (BT // 128) + tcc
                        nc.sync.dma_start(
                            y_d[t * 128:(t + 1) * 128, :], xr[:, t, :]
                        )

            pst_ctx.__exit__(None, None, None)

    nc.finalize()
    return nc


_NC_CACHE = {}


def _get_nc():
    if "nc" not in _NC_CACHE:
        _NC_CACHE["nc"] = build()
    return _NC_CACHE["nc"]


def _q8(x):
    return np.clip(np.asarray(x, np.float32), -240, 240).astype(NP8)


def kernel(**inputs):
    x = np.asarray(inputs["x"], dtype=np.float32)
    qkv_w = np.asarray(inputs["qkv_w"], dtype=np.float32)
    proj_w = np.asarray(inputs["proj_w"], dtype=np.float32)
    fc1_w = np.asarray(inputs["fc1_w"], dtype=np.float32)
    fc2_w = np.asarray(inputs["fc2_w"], dtype=np.float32)
    ln1_g = np.asarray(inputs["ln1_g"], dtype=np.float32)
    ln2_g = np.asarray(inputs["ln2_g"], dtype=np.float32)
    rel_pos_bias = np.asarray(inputs["rel_pos_bias"], dtype=np.float32)
    rel_pos_idx = np.asarray(inputs["rel_pos_idx"])

    for name in ("qkv_b", "proj_b", "fc1_b", "fc2_b", "ln1_b", "ln2_b"):
        assert not np.any(np.asarray(inputs[name])), f"nonzero {name} unsupported"

    wqkv = (ln1_g[:, None] * qkv_w).reshape(KC, 128, 3 * C).transpose(1, 0, 2)
    wqkv8 = _q8(64 * wqkv)
    wp = proj_w.reshape(KC, 128, C).transpose(1, 0, 2)
    wp8 = _q8(64 * wp)
    w1 = (ln2_g[:, None] * fc1_w).reshape(KC, 128, DFF).transpose(1, 0, 2)
    w1_hi = _q8(64 * w1)
    w1_lo = _q8(64 * w1 - w1_hi.astype(np.float32))
    w2 = fc2_w.reshape(KF, 128, C).transpose(1, 0, 2)   # [128, KF, C]
    w2_hi = _q8(64 * w2)
    w2_lo = _q8(64 * w2 - w2_hi.astype(np.float32))
    w2d = np.ascontiguousarray(np.stack([w2_hi, w2_lo], axis=2))  # [128,KF,2,C]

    # device multiplies by 8*I (DoubleRow pair) and exp applies scale 1/8,
    # so store a dual-fp8 split of B itself: [128, 2(hi/lo), 2(nk), H, q]
    Bm = rel_pos_bias[rel_pos_idx].reshape(N, N, H)          # [q, k, h]
    btf = np.ascontiguousarray(
        Bm.transpose(1, 2, 0).reshape(2, 128, H, N).transpose(1, 0, 2, 3)
    ).astype(np.float32)                                     # [128, 2(nk), H, q]
    bt_hi = _q8(btf)
    bt_lo = _q8(btf - bt_hi.astype(np.float32))
    bt = np.ascontiguousarray(np.stack([bt_hi, bt_lo], axis=1))

    nc = _get_nc()
    in_maps = []
    for c in range(NCORES):
        xs = np.ascontiguousarray(
            x[c * BLOC:(c + 1) * BLOC].reshape(TOK, C)
        ).astype(np.float32)
        in_maps.append(
            dict(x=xs, wqkv=wqkv8, wp=wp8, bt=bt, w1hi=w1_hi, w1lo=w1_lo, w2=w2d)
        )
    res = run_bass_kernel_spmd(nc, in_maps, core_ids=list(range(NCORES)))
    y = np.concatenate([res.results[c]["y"] for c in range(NCORES)], axis=0)
    return y.reshape(B, N, C).astype(np.float32)


# revision 24
# speedup vs baseline: 2.4230x; 1.0338x over previous
"""LITv1 transformer block on 8 TRN2 NeuronCores, data-parallel over batch.

v2: fp8 DoubleRow matmuls with error-feedback dual-fp8 operands.

Per-core layout (8 batches x 256 tokens):
- x/r1 residual stream resident in SBUF f32 (one 8MB buffer, updated in place)
- LN stats batched per phase (one Act Sqrt per phase -> few act-table loads)
- QKV: x-dual fp8 DoubleRow (xn hi+lo pairs in the moving slots, single-fp8
  weights broadcast stride-0 into the stationary slots) -> 0.25 cyc/k-tile
- attention internals bf16: S^T = K^T.T @ Q^T with the relative-position bias
  accumulated into PSUM via an 8*I identity matmul; exp on Act -> bf16 P;
  token-major AV with a ones-column for the softmax denominator
- proj: o-dual fp8 DoubleRow stationary, single-fp8 weights moving
- MLP: fc1 x-dual fp8 DoubleRow (xn2 hi+lo), single-fp8 W1; gelu -> fp8 h;
  fc2 w-dual fp8 DoubleRow (W2 hi+lo streamed from DRAM), h broadcast
- all transposes in bf16 via PE identity matmuls; dual-fp8 splits happen at
  the PSUM evacuation (hi = copy, lo = psum - hi via scalar_tensor_tensor)
"""
import sys

import numpy as np

sys.path.insert(0, "/opt/trn_rl_repo")

import ml_dtypes  # noqa: E402

import concourse.bass as bass  # noqa: E402
import concourse.mybir as mybir  # noqa: E402
import concourse.tile as tile  # noqa: E402
from concourse import bacc  # noqa: E402
from concourse.bass_utils import run_bass_kernel_spmd  # noqa: E402
from concourse.masks import make_identity  # noqa: E402

F32 = mybir.dt.float32
F8 = mybir.dt.float8e4
BF16 = mybir.dt.bfloat16
AF = mybir.ActivationFunctionType
ALU = mybir.AluOpType
PM = mybir.MatmulPerfMode

NP8 = ml_dtypes.float8_e4m3
NPB = ml_dtypes.bfloat16

B, N, C = 64, 256, 1024
H, DH = 16, 64
DFF = 4 * C
NCORES = 8
BLOC = B // NCORES          # 8 batches per core
TOK = BLOC * N              # 2048 tokens per core
NT = TOK // 128             # 16 token tiles of 128
KC = C // 128               # 8 contraction chunks of 128
KF = DFF // 128             # 32 dff chunks


def _pair(ap):
    """Broadcast a [128, X] AP to [128, 2, X] with stride-0 pair dim."""
    return ap.unsqueeze(1).broadcast_to([ap.shape[0], 2, ap.shape[-1]])


def build():
    nc = bacc.Bacc("TRN2")
    x_d = nc.dram_tensor("x", [TOK, C], F32, kind="ExternalInput")
    wqkv_d = nc.dram_tensor("wqkv", [128, KC, 3 * C], F8, kind="ExternalInput")
    wp_d = nc.dram_tensor("wp", [128, KC, C], F8, kind="ExternalInput")
    bt_d = nc.dram_tensor("bt", [128, 2, 2, H, N], F8, kind="ExternalInput")
    w1hi_d = nc.dram_tensor("w1hi", [128, KC, DFF], F8, kind="ExternalInput")
    w1lo_d = nc.dram_tensor("w1lo", [128, KC, DFF], F8, kind="ExternalInput")
    w2_d = nc.dram_tensor("w2", [128, KF, 2, C], F8, kind="ExternalInput")
    y_d = nc.dram_tensor("y", [TOK, C], F32, kind="ExternalOutput")

    with tile.TileContext(nc) as tc:
        with (
            tc.tile_pool(name="consts", bufs=1) as consts,
            tc.tile_pool(name="resid", bufs=1) as resid,
        ):
            ident_f = consts.tile([128, 128], F32)
            make_identity(nc, ident_f)
            identb = consts.tile([128, 128], BF16)
            nc.vector.tensor_copy(identb, ident_f)
            eight8 = consts.tile([128, 128], F8)
            nc.vector.tensor_scalar(eight8, ident_f, 8.0, None, ALU.mult)
            eps_sb = consts.tile([128, 1], F32)
            nc.vector.memset(eps_sb, 1e-5)

            xr = resid.tile([128, NT, C], F32)      # x then r1 then y, in place
            mvs = resid.tile([128, NT, 2], F32)     # LN1 mean/var
            rstds = resid.tile([128, NT], F32)
            mvs2 = resid.tile([128, NT, 2], F32)    # LN2 mean/var
            rstds2 = resid.tile([128, NT], F32)

            # ---------------- prologue: load x, LN1 stats --------------------
            pst_ctx = tc.tile_pool(name="pst", bufs=2)
            pst = pst_ctx.__enter__()
            sq = pst.tile([128, NT], F32, tag="sq")

            def ln1_stats(lo, hi_t):
                for t in range(lo, hi_t):
                    st = pst.tile([128, 2, 6], F32, tag="st", name="st")
                    xv = xr[:, t, :].rearrange("p (s f) -> p s f", s=2)
                    for s in range(2):
                        nc.vector.bn_stats(st[:, s, :], xv[:, s, :])
                    nc.vector.bn_aggr(mvs[:, t, :], st)
                nc.scalar.activation(
                    sq[:, lo:hi_t], mvs[:, lo:hi_t, 1], AF.Sqrt,
                    bias=eps_sb, scale=1.0,
                )
                nc.vector.reciprocal(rstds[:, lo:hi_t], sq[:, lo:hi_t])

            # first 4 x tiles unblock batch 0 quickly
            for t in range(4):
                nc.sync.dma_start(xr[:, t, :], x_d[t * 128:(t + 1) * 128, :])
            ln1_stats(0, 4)

            # ---------------- phase A: attention + proj ----------------------
            with (
                tc.tile_pool(name="paw", bufs=1) as paw,
                tc.tile_pool(name="pa1", bufs=2) as pa1,
                tc.tile_pool(name="pa2", bufs=2) as pa2,
                tc.tile_pool(name="pae", bufs=16) as pae,
                tc.tile_pool(name="psT", bufs=2, space="PSUM") as psT,
                tc.tile_pool(name="psQK", bufs=1, space="PSUM") as psQK,
                tc.tile_pool(name="psMM", bufs=1, space="PSUM") as psMM,
                tc.tile_pool(name="psS", bufs=2, space="PSUM") as psS,
                tc.tile_pool(name="psAV", bufs=2, space="PSUM") as psAV,
            ):
                # DMA order: wqkv before remaining x tiles so batch 0's QKV
                # isn't blocked; bt before wp (bias needed earlier than proj)
                wqkv_sb = paw.tile([128, KC, 3 * C], F8)
                nc.sync.dma_start(wqkv_sb, wqkv_d[:])
                bt_sb = paw.tile([128, 2, 2, H, N], F8)
                nc.sync.dma_start(bt_sb, bt_d[:])
                wp_sb = paw.tile([128, KC, C], F8)
                nc.sync.dma_start(wp_sb, wp_d[:])
                for t in range(4, NT):
                    nc.sync.dma_start(xr[:, t, :], x_d[t * 128:(t + 1) * 128, :])

                def tp4(pool_tag, srcs, ident):
                    """4 transposes into one PSUM bank as ONE accumulation
                    group (hardware zeroes PSUM lazily per 2KB region; separate
                    groups in one region would wipe earlier sub-tiles)."""
                    tp = psT.tile([128, 4, 128], BF16, tag=pool_tag)
                    for j, src in enumerate(srcs):
                        nc.tensor.matmul(
                            tp[:, j, :], src, ident, is_transpose=True,
                            start=(j == 0), stop=(j == 3),
                        )
                    return tp

                def stage1(b):
                    """LN1 norm/transpose/dual-evac + QKV(q,k) matmuls."""
                    xn16 = pa1.tile([128, 2, C], BF16, tag="xn16", name="xn16")
                    xnT = pa1.tile([128, KC, 2, N], F8, tag="xnT", name="xnT")
                    for i in range(2):
                        t = 2 * b + i
                        nc.vector.tensor_scalar(
                            xn16[:, i, :], xr[:, t, :],
                            mvs[:, t, 0:1], rstds[:, t:t + 1],
                            ALU.subtract, ALU.mult,
                        )
                        for cg in range(2):
                            tp = tp4("tp", [
                                xn16[:, i, (cg * 4 + j) * 128:(cg * 4 + j + 1) * 128]
                                for j in range(4)
                            ], identb)
                            hi = xnT[:, cg * 4:(cg + 1) * 4, 0, i * 128:(i + 1) * 128]
                            nc.scalar.copy(hi, tp)
                            nc.vector.scalar_tensor_tensor(
                                xnT[:, cg * 4:(cg + 1) * 4, 1, i * 128:(i + 1) * 128],
                                tp, 1.0, hi, ALU.mult, ALU.subtract,
                            )

                    qkT = pa2.tile([128, 2 * KC, N], BF16, tag="qkT", name="qkT")
                    for fp in range(KC):       # pairs of 128-feat chunks (Q,K)
                        qp = psQK.tile([128, 2, N], F32, tag="qk", name="qp")
                        for s in range(2):
                            fo = 2 * fp + s
                            for k in range(KC):
                                nc.tensor.matmul(
                                    qp[:, s, :],
                                    _pair(wqkv_sb[:, k, fo * 128:(fo + 1) * 128]),
                                    xnT[:, k, :, :],
                                    start=(s == 0 and k == 0),
                                    stop=(s == 1 and k == KC - 1),
                                    perf_mode=PM.DoubleRow,
                                )
                        nc.scalar.activation(
                            qkT[:, 2 * fp:2 * fp + 2, :], qp, AF.Copy,
                            scale=1.0 / 64,
                        )
                    return xnT, qkT

                def stage2(b, xnT, qkT):
                    """S^T + dual-fp8 bias + exp -> bf16 P, with the V matmuls
                    interleaved so PE has work while Act drains the exps."""
                    es = []
                    v_sb = pa2.tile([128, 2, H, DH + 1], BF16, tag="v", name="v_sb")
                    nc.vector.memset(v_sb[:, :, :, DH:DH + 1], 1.0)

                    def s_head(h):
                        sp = psS.tile([128, 2, N], F32, tag="sp", name="sp")
                        r0 = (h % 2) * 64
                        for nk in range(2):
                            nc.tensor.matmul(
                                sp[:, nk, :], _pair(eight8),
                                bt_sb[:, :, nk, h, :],
                                start=(nk == 0), stop=False,
                                perf_mode=PM.DoubleRow,
                            )
                            nc.tensor.matmul(
                                sp[:, nk, :],
                                qkT[r0:r0 + 64, KC + h // 2, nk * 128:(nk + 1) * 128],
                                qkT[r0:r0 + 64, h // 2, :],
                                start=False, stop=(nk == 1),
                            )
                        e = pae.tile([128, 2, N], BF16, tag="e", name="e")
                        nc.scalar.activation(e, sp, AF.Exp, bias=0.0, scale=0.125)
                        es.append(e)

                    def v_half(t):
                        for vh in range(2):    # halves of V feature dim
                            vp = psMM.tile([128, 2, N], F32, tag="mm", name="vp")
                            for s in range(2):
                                vq = 2 * vh + s
                                for k in range(KC):
                                    nc.tensor.matmul(
                                        vp[:, s, :],
                                        xnT[:, k, :, t * 128:(t + 1) * 128],
                                        _pair(wqkv_sb[:, k, 2 * C + vq * 256:2 * C + (vq + 1) * 256]),
                                        start=(s == 0 and k == 0),
                                        stop=(s == 1 and k == KC - 1),
                                        perf_mode=PM.DoubleRow,
                                    )
                            nc.vector.tensor_scalar(
                                v_sb[:, t, vh * 8:(vh + 1) * 8, 0:DH],
                                vp.rearrange("p s (a d) -> p (s a) d", d=DH),
                                1.0 / 64, None, ALU.mult,
                            )

                    for h in range(H // 2):
                        s_head(h)
                    v_half(0)
                    for h in range(H // 2, H):
                        s_head(h)
                    v_half(1)
                    return es, v_sb

                def stage3(b, es, v_sb):
                    """AV + normalize, O transpose, proj + residual, LN2 stats."""
                    o_sb = pa1.tile([128, 2, H, DH], BF16, tag="o", name="o_sb")
                    rd = pa1.tile([128, 2, H], F32, tag="rd", name="rd")
                    oT = pa1.tile([128, KC, 2, N], F8, tag="oT", name="oT")
                    for qc in range(2):
                        for hg in range(4):
                            av = psAV.tile([128, 4, DH + 1], F32, tag="av", name="av")
                            for hh in range(4):
                                h = hg * 4 + hh
                                for nk in range(2):
                                    nc.tensor.matmul(
                                        av[:, hh, :],
                                        es[h][:, nk, qc * 128:(qc + 1) * 128],
                                        v_sb[:, nk, h, :],
                                        start=(hh == 0 and nk == 0),
                                        stop=(hh == 3 and nk == 1),
                                    )
                            nc.vector.reciprocal(
                                rd[:, qc, hg * 4:(hg + 1) * 4], av[:, :, DH]
                            )
                            for hh in range(4):
                                h = hg * 4 + hh
                                nc.vector.tensor_scalar(
                                    o_sb[:, qc, h, :], av[:, hh, 0:DH],
                                    rd[:, qc, h:h + 1], None, ALU.mult,
                                )
                        for cg in range(2):
                            tp = tp4("tp", [
                                o_sb[:, qc, 2 * (cg * 4 + j):2 * (cg * 4 + j) + 2, :]
                                .rearrange("p a d -> p (a d)")
                                for j in range(4)
                            ], identb)
                            hi = oT[:, cg * 4:(cg + 1) * 4, 0, qc * 128:(qc + 1) * 128]
                            nc.scalar.copy(hi, tp)
                            nc.vector.scalar_tensor_tensor(
                                oT[:, cg * 4:(cg + 1) * 4, 1, qc * 128:(qc + 1) * 128],
                                tp, 1.0, hi, ALU.mult, ALU.subtract,
                            )

                    for t in range(2):
                        for ch in range(2):    # output halves of 512
                            pp = psMM.tile([128, 2, N], F32, tag="mm", name="pp")
                            for s in range(2):
                                cq = 2 * ch + s
                                for k in range(KC):
                                    nc.tensor.matmul(
                                        pp[:, s, :],
                                        oT[:, k, :, t * 128:(t + 1) * 128],
                                        _pair(wp_sb[:, k, cq * 256:(cq + 1) * 256]),
                                        start=(s == 0 and k == 0),
                                        stop=(s == 1 and k == KC - 1),
                                        perf_mode=PM.DoubleRow,
                                    )
                            tt = 2 * b + t
                            nc.vector.scalar_tensor_tensor(
                                xr[:, tt, ch * 512:(ch + 1) * 512],
                                pp.rearrange("p s n -> p (s n)"), 1.0 / 64,
                                xr[:, tt, ch * 512:(ch + 1) * 512],
                                ALU.mult, ALU.add,
                            )
                        tt = 2 * b + t
                        st2 = pa1.tile([128, 2, 6], F32, tag="st2a", name="st2")
                        rv = xr[:, tt, :].rearrange("p (s f) -> p s f", s=2)
                        for s in range(2):
                            nc.vector.bn_stats(st2[:, s, :], rv[:, s, :])
                        nc.vector.bn_aggr(mvs2[:, tt, :], st2)

                # software pipeline: stage3(b-1) slots between stage1(b)
                # and stage2(b) so PE never waits on Act exps / DVE evacs
                carry = None
                for b in range(BLOC):
                    xnT, qkT = stage1(b)
                    if b == 0:
                        ln1_stats(4, NT)
                    if carry is not None:
                        stage3(b - 1, *carry)
                    carry = stage2(b, xnT, qkT)
                stage3(BLOC - 1, *carry)

            # ---------------- phase B: MLP ----------------------------------
            with (
                tc.tile_pool(name="pbst", bufs=2) as pbst,
                tc.tile_pool(name="pbw", bufs=2) as pbw,
                tc.tile_pool(name="pbh", bufs=1) as pbh,
                tc.tile_pool(name="pb1", bufs=2) as pb1,
                tc.tile_pool(name="psT2", bufs=2, space="PSUM") as psT2,
                tc.tile_pool(name="psF1", bufs=3, space="PSUM") as psF1,
                tc.tile_pool(name="psF2", bufs=2, space="PSUM") as psF2,
            ):
                # LN2 stats were computed per-batch in phase A
                sq2 = pbst.tile([128, NT], F32, tag="sq2")
                nc.scalar.activation(sq2, mvs2[:, :, 1], AF.Sqrt, bias=eps_sb, scale=1.0)
                nc.vector.reciprocal(rstds2, sq2)

                NB = 2
                BT = TOK // NB          # 1024 tokens per block
                for blk in range(NB):
                    xnT2 = pbh.tile([128, KC, 2, BT], F8, tag="xnT2")
                    hT = pbh.tile([128, KF, BT], F8, tag="hT")
                    for tt in range(BT // 128):
                        t = blk * (BT // 128) + tt
                        xn2 = pb1.tile([128, C], BF16, tag="xn2")
                        nc.vector.tensor_scalar(
                            xn2, xr[:, t, :], mvs2[:, t, 0:1], rstds2[:, t:t + 1],
                            ALU.subtract, ALU.mult,
                        )
                        for cg in range(2):
                            tp = psT2.tile([128, 4, 128], BF16, tag="tp2")
                            for j in range(4):
                                nc.tensor.matmul(
                                    tp[:, j, :],
                                    xn2[:, (cg * 4 + j) * 128:(cg * 4 + j + 1) * 128],
                                    identb, is_transpose=True,
                                    start=(j == 0), stop=(j == 3),
                                )
                            hi = xnT2[:, cg * 4:(cg + 1) * 4, 0, tt * 128:(tt + 1) * 128]
                            nc.scalar.copy(hi, tp)
                            nc.vector.scalar_tensor_tensor(
                                xnT2[:, cg * 4:(cg + 1) * 4, 1, tt * 128:(tt + 1) * 128],
                                tp, 1.0, hi, ALU.mult, ALU.subtract,
                            )

                    # fc1: both-dual via 3 single-DR passes
                    # (w_hi@x_hi + w_lo@x_hi + w_hi@x_lo), gelu -> fp8 hT
                    for sl in range(8):     # dff slices of 512
                        w1hc = pbw.tile([128, KC, 512], F8, tag="w1hc")
                        nc.sync.dma_start(w1hc, w1hi_d[:, :, sl * 512:(sl + 1) * 512])
                        w1lc = pbw.tile([128, KC, 512], F8, tag="w1lc")
                        nc.sync.dma_start(w1lc, w1lo_d[:, :, sl * 512:(sl + 1) * 512])
                        for dc in range(4):
                            for tc2 in range(2):
                                fp1 = psF1.tile([128, 2, 256], F32, tag="f1")
                                passes = [(w1hc, 0), (w1lc, 0), (w1hc, 1)]
                                for th in range(2):
                                    tq = 2 * tc2 + th
                                    for pi, (wt, xi) in enumerate(passes):
                                        for kp in range(KC // 2):
                                            nc.tensor.matmul(
                                                fp1[:, th, :],
                                                wt[:, 2 * kp:2 * kp + 2,
                                                   dc * 128:(dc + 1) * 128],
                                                xnT2[:, 2 * kp:2 * kp + 2, xi,
                                                     tq * 256:(tq + 1) * 256],
                                                start=(th == 0 and pi == 0 and kp == 0),
                                                stop=(th == 1 and pi == 2
                                                      and kp == KC // 2 - 1),
                                                perf_mode=PM.DoubleRow,
                                            )
                                nc.scalar.activation(
                                    hT[:, sl * 4 + dc, tc2 * 512:(tc2 + 1) * 512],
                                    fp1.rearrange("p a b -> p (a b)"),
                                    AF.Gelu_apprx_tanh, scale=1.0 / 64,
                                )

                    # fc2: w-dual DoubleRow + residual -> y
                    for co in range(4):     # output quarters of 256
                        w2c = pbw.tile([128, KF, 2, 256], F8, tag="w2c")
                        nc.sync.dma_start(w2c, w2_d[:, :, :, co * 256:(co + 1) * 256])
                        for tcc in range(BT // 128):
                            t = blk * (BT // 128) + tcc
                            fp2 = psF2.tile([128, 256], F32, tag="f2")
                            for k in range(KF):
                                nc.tensor.matmul(
                                    fp2,
                                    _pair(hT[:, k, tcc * 128:(tcc + 1) * 128]),
                                    w2c[:, k, :, :],
                                    start=(k == 0), stop=(k == KF - 1),
                                    perf_mode=PM.DoubleRow,
                                )
                            nc.vector.scalar_tensor_tensor(
                                xr[:, t, co * 256:(co + 1) * 256], fp2, 1.0 / 64,
                                xr[:, t, co * 256:(co + 1) * 256],
                                ALU.mult, ALU.add,
                            )
                    for tcc in range(BT // 128):
                        t = blk * (BT // 128) + tcc
                        nc.sync.dma_start(
                            y_d[t * 128:(t + 1) * 128, :], xr[:, t, :]
                        )

            pst_ctx.__exit__(None, None, None)

    nc.finalize()
    return nc


_NC_CACHE = {}


def _get_nc():
    if "nc" not in _NC_CACHE:
        _NC_CACHE["nc"] = build()
    return _NC_CACHE["nc"]


def _q8(x):
    return np.clip(np.asarray(x, np.float32), -240, 240).astype(NP8)


def kernel(**inputs):
    x = np.asarray(inputs["x"], dtype=np.float32)
    qkv_w = np.asarray(inputs["qkv_w"], dtype=np.float32)
    proj_w = np.asarray(inputs["proj_w"], dtype=np.float32)
    fc1_w = np.asarray(inputs["fc1_w"], dtype=np.float32)
    fc2_w = np.asarray(inputs["fc2_w"], dtype=np.float32)
    ln1_g = np.asarray(inputs["ln1_g"], dtype=np.float32)
    ln2_g = np.asarray(inputs["ln2_g"], dtype=np.float32)
    rel_pos_bias = np.asarray(inputs["rel_pos_bias"], dtype=np.float32)
    rel_pos_idx = np.asarray(inputs["rel_pos_idx"])

    for name in ("qkv_b", "proj_b", "fc1_b", "fc2_b", "ln1_b", "ln2_b"):
        assert not np.any(np.asarray(inputs[name])), f"nonzero {name} unsupported"

    wqkv = (ln1_g[:, None] * qkv_w).reshape(KC, 128, 3 * C).transpose(1, 0, 2)
    wqkv8 = _q8(64 * wqkv)
    wp = proj_w.reshape(KC, 128, C).transpose(1, 0, 2)
    wp8 = _q8(64 * wp)
    w1 = (ln2_g[:, None] * fc1_w).reshape(KC, 128, DFF).transpose(1, 0, 2)
    w1_hi = _q8(64 * w1)
    w1_lo = _q8(64 * w1 - w1_hi.astype(np.float32))
    w2 = fc2_w.reshape(KF, 128, C).transpose(1, 0, 2)   # [128, KF, C]
    w2_hi = _q8(64 * w2)
    w2_lo = _q8(64 * w2 - w2_hi.astype(np.float32))
    w2d = np.ascontiguousarray(np.stack([w2_hi, w2_lo], axis=2))  # [128,KF,2,C]

    # device multiplies by 8*I (DoubleRow pair) and exp applies scale 1/8,
    # so store a dual-fp8 split of B itself: [128, 2(hi/lo), 2(nk), H, q]
    Bm = rel_pos_bias[rel_pos_idx].reshape(N, N, H)          # [q, k, h]
    btf = np.ascontiguousarray(
        Bm.transpose(1, 2, 0).reshape(2, 128, H, N).transpose(1, 0, 2, 3)
    ).astype(np.float32)                                     # [128, 2(nk), H, q]
    bt_hi = _q8(btf)
    bt_lo = _q8(btf - bt_hi.astype(np.float32))
    bt = np.ascontiguousarray(np.stack([bt_hi, bt_lo], axis=1))

    nc = _get_nc()
    in_maps = []
    for c in range(NCORES):
        xs = np.ascontiguousarray(
            x[c * BLOC:(c + 1) * BLOC].reshape(TOK, C)
        ).astype(np.float32)
        in_maps.append(
            dict(x=xs, wqkv=wqkv8, wp=wp8, bt=bt, w1hi=w1_hi, w1lo=w1_lo, w2=w2d)
        )
    res = run_bass_kernel_spmd(nc, in_maps, core_ids=list(range(NCORES)))
    y = np.concatenate([res.results[c]["y"] for c in range(NCORES)], axis=0)
    return y.reshape(B, N, C).astype(np.float32)
